# revision 12
# baseline (speedup 1.0000x reference)
"""Multi-head attention (B=16, N=1024, D=1024, H=8, dh=128) on 8 trn2 cores.

Strategy: data-parallel over batch (2 batches/core), fp32r matmuls.
Per batch on each core:
  phase 1 (per 2-head group g): Q^T_g, K^T_g (head-transposed: dh on
    partitions) and V_g (natural) via fp32r matmuls from x^T (host-side
    pre-transposed) and streamed weight slices.
  phase 2 (per head, per 512-wide q chunk): S^T = K_h^T.T @ Q_h^T (k on
    partitions), E^T = exp(norm*S^T) on ACT, heads^T += V_h.T @ E^T, and
    R = colsum(E^T) via DVE/Pool pairwise adds, then one all-ones 128x128
    matmul that yields R already broadcast to every partition; 1/R via
    a fast 128-lane reciprocal, applied while copying heads^T to SBUF.
  phase 3: out = (heads_norm) @ Wo in natural layout (+ bv@Wo row via a
    K=1 matmul when biases are nonzero).

Scheduling: Wo is resident in SBUF (loaded once), startup DMAs are
ordered so the first projection matmuls start as early as possible,
attention units are queued q-chunk-major so the final batch's output
projection can interleave with the attention drain, and PSUM->SBUF
copies run on the otherwise-idle Pool engine.
"""

import numpy as np

import concourse.bass as bass
import concourse.mybir as mybir
import concourse.tile as tile
from concourse import bacc
from concourse.bass_utils import run_bass_kernel_spmd

N_CORES = 8
B = 16
BPC = B // N_CORES      # batches per core
N = 1024                # sequence length
D = 1024                # model dim
H = 8                   # heads
DH = 128                # head dim
P = 128
DB = D // P             # 8 contraction blocks
GH = 2                  # heads per group
G = H // GH             # 4 groups
GW = GH * DH            # 256: e-width per group
NC2 = N // 512          # 2 n-chunks of 512
NORM = 1.0 / np.sqrt(DH)

F32 = mybir.dt.float32
F32R = mybir.dt.float32r


def r(ap):
    return ap


def build_nc(has_bias=True):
    nc = bacc.Bacc()
    xT = nc.declare_dram_parameter("xT", [BPC, D, N], F32R, isOutput=False)
    Wq = nc.declare_dram_parameter("Wq", [D, D], F32R, isOutput=False)
    Wk = nc.declare_dram_parameter("Wk", [D, D], F32R, isOutput=False)
    Wv = nc.declare_dram_parameter("Wv", [D, D], F32R, isOutput=False)
    Wo = nc.declare_dram_parameter("Wo", [D, D], F32R, isOutput=False)
    bq = nc.declare_dram_parameter("bq", [D], F32, isOutput=False)
    bk = nc.declare_dram_parameter("bk", [D], F32, isOutput=False)
    bv = nc.declare_dram_parameter("bv", [D], F32R, isOutput=False)
    out = nc.declare_dram_parameter("out", [BPC, N, D], F32, isOutput=True)

    ws = [Wq, Wk, Wv]

    with tile.TileContext(nc) as tc:
        with tc.tile_pool(name="big", bufs=1) as big, \
             tc.tile_pool(name="wp", bufs=1) as wp, \
             tc.tile_pool(name="work", bufs=1) as work, \
             tc.tile_pool(name="small", bufs=1) as small, \
             tc.tile_pool(name="ps", bufs=1, space="PSUM") as ps:

            # constants / biases
            bq_col = small.tile([P, DB], F32, name="bq_col")
            bk_col = small.tile([P, DB], F32, name="bk_col")
            bv_col = small.tile([P, DB], F32R, name="bv_col")
            nc.sync.dma_start(out=bq_col, in_=bq.rearrange("(eb p) -> p eb", p=P))
            nc.sync.dma_start(out=bk_col, in_=bk.rearrange("(eb p) -> p eb", p=P))
            nc.sync.dma_start(out=bv_col, in_=bv.rearrange("(eb p) -> p eb", p=P))
            ones128_f32 = small.tile([P, P], F32, name="ones128_f32")
            nc.vector.memset(ones128_f32, 1.0)
            ones128 = small.tile([P, P], F32R, name="ones128")
            nc.vector.tensor_copy(ones128, ones128_f32)
            if has_bias:
                ones_row_f32 = small.tile([1, P], F32, name="ones_row_f32")
                nc.vector.memset(ones_row_f32, 1.0)
                ones_row = small.tile([1, P], F32R, name="ones_row")
                nc.vector.tensor_copy(ones_row, ones_row_f32)
                c_sb = small.tile([1, NC2, 512], F32R, name="c_sb")

            # persistent Wo: [128, eb, o] loaded once, shared by both batches
            wo_sb = wp.tile([P, DB, N], F32R, name="wo_sb", tag="wo_sb")
            wo_src = Wo.rearrange("(eb p) o -> p eb o", p=P)

            def issue_wo_dma():
                for q in range(4):
                    nc.sync.dma_start(out=wo_sb[:, 2 * q:2 * q + 2, :],
                                      in_=wo_src[:, 2 * q:2 * q + 2, :])

            def alloc_wgt(b, g):
                gsfx = f"_b{b}_g{g}"
                wgt = {}
                for wname in ("wq", "wk", "wv"):
                    wgt[wname] = wp.tile([P, DB, GW], F32R, name=f"{wname}{gsfx}",
                                         tag="wg", bufs=2)
                return wgt

            def issue_wgt_dma(wgt, g, names=("wq", "wk", "wv")):
                e0 = g * GW
                for wi, wname in enumerate(("wq", "wk", "wv")):
                    if wname not in names:
                        continue
                    wt = wgt[wname]
                    src = ws[wi].rearrange("(db p) e -> p db e", p=P)
                    nc.sync.dma_start(out=wt[:, 0:DB // 2, :],
                                      in_=src[:, 0:DB // 2, e0:e0 + GW])
                    nc.sync.dma_start(out=wt[:, DB // 2:, :],
                                      in_=src[:, DB // 2:, e0:e0 + GW])

            def emit_proj_unit(b, g, kind, idx, xt, wgt, qTg, kTg, vg):
                """Emit one psum accumulation group of phase 1."""
                gsfx = f"_b{b}_g{g}"
                e0 = g * GW
                if kind in ("q", "k"):
                    dst, wt, bcol = ((qTg, wgt["wq"], bq_col) if kind == "q"
                                     else (kTg, wgt["wk"], bk_col))
                    eb, nch = divmod(idx, NC2)
                    acc = ps.tile([P, 512], F32, tag="pj", bufs=2,
                                  name=f"p{kind}{gsfx}_{eb}_{nch}")
                    for db in range(DB):
                        nc.tensor.matmul(
                            acc,
                            r(wt[:, db, eb * P:(eb + 1) * P]),
                            r(xt[:, db, nch * 512:(nch + 1) * 512]),
                            start=(db == 0), stop=(db == DB - 1))
                    ebg = (e0 // P) + eb
                    if has_bias:
                        nc.vector.tensor_scalar_add(
                            dst[:, eb, nch * 512:(nch + 1) * 512],
                            acc, bcol[:, ebg:ebg + 1])
                    else:
                        nc.vector.tensor_copy(
                            dst[:, eb, nch * 512:(nch + 1) * 512], acc)
                else:  # "v"
                    nb = idx
                    accv = ps.tile([P, 512], F32, tag="pj", bufs=2,
                                   name=f"pv{gsfx}_{nb}")
                    for db in range(DB):
                        nc.tensor.matmul(
                            accv[:, :GW],
                            r(xt[:, db, nb * P:(nb + 1) * P]),
                            r(wgt["wv"][:, db, :]),
                            start=(db == 0), stop=(db == DB - 1))
                    nc.vector.tensor_copy(vg[:, nb, :], accv[:, :GW])

            def make_phase3(b, hT):
                """Return list of emit closures: [c-prelude?] + 16 po units
                ordered so the first 8 only need q rows < 512 (qc=0)."""
                units = []
                sfx = f"_b{b}"

                if b == 0 and has_bias:
                    def emit_c():
                        for oc in range(NC2):
                            pc = ps.tile([1, 512], F32, tag="pj", bufs=2,
                                         name=f"pc_{oc}")
                            for eb in range(DB):
                                nc.tensor.matmul(pc, r(bv_col[:, eb:eb + 1]),
                                                 r(wo_sb[:, eb, oc * 512:(oc + 1) * 512]),
                                                 start=(eb == 0), stop=(eb == DB - 1))
                            nc.vector.tensor_copy(c_sb[:, oc, :], pc)
                    units.append(emit_c)

                def make_po(oc, nb):
                    def emit():
                        po = ps.tile([P, 512], F32, tag="pj", bufs=2,
                                     name=f"po{sfx}_{oc}_{nb}")
                        for eb in range(H):
                            nc.tensor.matmul(
                                po,
                                r(hT[:, eb, nb * P:(nb + 1) * P]),
                                r(wo_sb[:, eb, oc * 512:(oc + 1) * 512]),
                                start=(eb == 0),
                                stop=(not has_bias and eb == H - 1))
                        if has_bias:
                            nc.tensor.matmul(po, r(ones_row), r(c_sb[:, oc, :]),
                                             start=False, stop=True)
                        osb = work.tile([P, 512], F32, name=f"o{sfx}_{oc}_{nb}",
                                        tag="osb", bufs=1)
                        nc.scalar.activation(osb, po,
                                             mybir.ActivationFunctionType.Copy)
                        nc.sync.dma_start(
                            out=out[b, nb * P:(nb + 1) * P, oc * 512:(oc + 1) * 512],
                            in_=osb)
                    return emit

                for nb in range(DB // 2):       # q rows < 512 only
                    for oc in range(NC2):
                        units.append(make_po(oc, nb))
                for nb in range(DB // 2, DB):   # q rows >= 512
                    for oc in range(NC2):
                        units.append(make_po(oc, nb))
                return units

            def make_attn(g, hh, qc, qTg, kTg, vg, hT, b):
                """Split attention unit: (emit_scores, emit_av)."""
                h = g * GH + hh
                asfx = f"_b{b}_h{h}_q{qc}"
                st = {}
                add = mybir.AluOpType.add

                def emit_scores():
                    eT = work.tile([P, 4, 1024], F32R, name=f"eT{asfx}",
                                   tag="eT", bufs=(1 if has_bias else 2))
                    st["eT"] = eT
                    for j in range(4):
                        # scores for kb=2j, 2j+1 into one 2-bank tile
                        sp = ps.tile([P, 1024], F32, tag="spair", bufs=2,
                                     name=f"sp{asfx}_{j}")
                        for half in range(2):
                            kb = 2 * j + half
                            nc.tensor.matmul(
                                sp[:, half * 512:(half + 1) * 512],
                                r(kTg[:, hh, kb * P:(kb + 1) * P]),
                                r(qTg[:, hh, qc * 512:(qc + 1) * 512]),
                                start=True, stop=True)
                        nc.scalar.activation(
                            eT[:, j, :], sp,
                            mybir.ActivationFunctionType.Exp,
                            scale=float(NORM))

                def emit_av():
                    eT = st["eT"]
                    # heads^T (unnormalized): [dv(128) x q(512)]
                    pav = ps.tile([P, 512], F32, tag="pav", bufs=1,
                                  name=f"pav{asfx}")
                    for j in range(4):
                        for half in range(2):
                            kb = 2 * j + half
                            nc.tensor.matmul(
                                pav,
                                r(vg[:, kb, hh * DH:(hh + 1) * DH]),
                                r(eT[:, j, half * 512:(half + 1) * 512]),
                                start=(kb == 0), stop=(kb == DB - 1))

                    # R = col-sum of E^T: pairwise adds split DVE/Pool
                    tA = work.tile([P, 1024], F32R, name=f"tA{asfx}", tag="tA", bufs=1)
                    tB = work.tile([P, 1024], F32R, name=f"tB{asfx}", tag="tB", bufs=1)
                    rp = work.tile([P, 512], F32R, name=f"rp{asfx}", tag="rp", bufs=1)
                    nc.vector.tensor_tensor(tA, eT[:, 0, :], eT[:, 1, :], add)
                    nc.vector.tensor_tensor(tB, eT[:, 2, :], eT[:, 3, :], add)
                    nc.vector.tensor_tensor(tA, tA, tB, add)
                    nc.vector.tensor_tensor(rp, tA[:, 0:512], tA[:, 512:1024], add)
                    # colsum of rp, broadcast to all partitions, in
                    # one matmul: every row of ones128.T @ rp is R
                    pbc = ps.tile([P, 512], F32, tag="pnorm", bufs=1, name=f"pbc{asfx}")
                    nc.tensor.matmul(pbc, r(ones128), r(rp),
                                     start=True, stop=True)
                    # 1/R at full 128-lane width (approx + one NR pass)
                    # (scratch shares the rp slot: rp's only reader, the pbc
                    # matmul, always precedes the reciprocal that writes it)
                    scratch = work.tile([P, 512], F32, name=f"sc{asfx}",
                                        tag="rp", bufs=1)
                    binv = work.tile([P, 512], F32, name=f"binv{asfx}",
                                     tag="binv", bufs=1)
                    nc.vector.reciprocal_approx_accurate(binv, pbc, scratch)
                    nc.vector.tensor_tensor(
                        hT[:, h, qc * 512:(qc + 1) * 512], pav, binv,
                        mybir.AluOpType.mult)

                return emit_scores, emit_av

            # attention units and the previous batch's output projection are
            # emitted interleaved with later projection units so PE always
            # has ready matmuls during exp/epilogue waits
            attn_queue = []
            pending_phase3 = None

            for b in range(BPC):
                sfx = f"_b{b}"
                xt = big.tile([P, DB, N], F32R, name=f"xt{sfx}", tag="xt")
                xsrc = xT[b].rearrange("(db p) n -> p db n", p=P)

                wgt0 = None
                if b == 0:
                    # startup: wq for group 0 must land before anything else
                    # so the first projection matmuls can begin immediately
                    wgt0 = alloc_wgt(0, 0)
                    issue_wgt_dma(wgt0, 0, names=("wq",))
                for db in range(DB):
                    nc.sync.dma_start(out=xt[:, db, :], in_=xsrc[:, db, :])
                if b == 0:
                    issue_wgt_dma(wgt0, 0, names=("wk", "wv"))

                hT = None

                for g in range(G):
                    # ---- weight slices for this group: [128, db, GW]
                    if b == 0 and g == 0:
                        wgt = wgt0
                    else:
                        wgt = alloc_wgt(b, g)
                        issue_wgt_dma(wgt, g)
                    if b == 0 and g == 1:
                        issue_wo_dma()

                    qTg = work.tile([P, GH, N], F32R, name=f"qT{sfx}_g{g}", tag="qTg", bufs=2)
                    kTg = work.tile([P, GH, N], F32R, name=f"kT{sfx}_g{g}", tag="kTg", bufs=2)
                    vg = work.tile([P, DB, GW], F32R, name=f"v{sfx}_g{g}", tag="vg", bufs=2)

                    # 16 proj units: 4 Q, 4 K, 8 V; interleave with up to 4
                    # pending attention units (1 attention per 4 proj units)
                    units = ([("q", i) for i in range(GH * NC2)]
                             + [("k", i) for i in range(GH * NC2)]
                             + [("v", i) for i in range(DB)])
                    for ui, (kind, idx) in enumerate(units):
                        emit_proj_unit(b, g, kind, idx, xt, wgt, qTg, kTg, vg)
                        if ui % 4 == 1 and attn_queue:
                            s_fn, a_fn = attn_queue.pop(0)
                            s_fn(); a_fn()
                    while attn_queue:
                        s_fn, a_fn = attn_queue.pop(0)
                        s_fn(); a_fn()
                    if pending_phase3 is not None:
                        for u in pending_phase3:
                            u()
                        pending_phase3 = None
                    if hT is None:
                        hT = big.tile([P, H, N], F32R, name=f"hT{sfx}", tag="hT")

                    # ---- queue attention for the heads of this group,
                    # q-chunk-major so all heads' qc=0 results land first
                    for qc in range(NC2):
                        for hh in range(GH):
                            attn_queue.append(
                                make_attn(g, hh, qc, qTg, kTg, vg, hT, b))

                if b < BPC - 1:
                    # phase 3 of this batch is deferred: it is emitted after
                    # the next batch's first projection group so its matmuls
                    # overlap the last attention units
                    pending_phase3 = make_phase3(b, hT)
                else:
                    # final batch: interleave the last group's attention with
                    # the output projection so PE keeps working through the
                    # exp chains of the final units
                    p3 = make_phase3(b, hT)
                    pre = p3[:-16]        # c-prelude if present (b==0 case)
                    po = p3[-16:]         # po[:8] need qc=0 only
                    drain = list(attn_queue)
                    attn_queue = []
                    # qc=0 units (first half of queue): run normally
                    for s_fn, a_fn in drain[:-2]:
                        s_fn(); a_fn()
                    for u in pre:
                        u()
                    # last two (qc=1) units: fill exp latency with po units
                    (s0, a0), (s1, a1) = drain[-2:]
                    s0()
                    po[0](); po[1]()
                    a0()
                    s1()
                    po[2](); po[3]()
                    a1()
                    for u in po[4:]:
                        u()

            # tail: drain remaining attention, then the last output projection
            while attn_queue:
                s_fn, a_fn = attn_queue.pop(0)
                s_fn(); a_fn()
            if pending_phase3 is not None:
                for u in pending_phase3:
                    u()

    nc.compile()
    return nc


_NC_CACHE = {}


def _get_nc(has_bias):
    if has_bias not in _NC_CACHE:
        _NC_CACHE[has_bias] = build_nc(has_bias)
    return _NC_CACHE[has_bias]


def make_in_maps(x, Wq, bq, Wk, bk, Wv, bv, Wo):
    x = np.asarray(x, dtype=np.float32)
    in_maps = []
    shared = {
        "Wq": np.ascontiguousarray(Wq, dtype=np.float32),
        "Wk": np.ascontiguousarray(Wk, dtype=np.float32),
        "Wv": np.ascontiguousarray(Wv, dtype=np.float32),
        "Wo": np.ascontiguousarray(Wo, dtype=np.float32),
        "bq": np.ascontiguousarray(bq, dtype=np.float32),
        "bk": np.ascontiguousarray(bk, dtype=np.float32),
        "bv": np.ascontiguousarray(bv, dtype=np.float32),
    }
    for c in range(N_CORES):
        xc = x[c * BPC:(c + 1) * BPC]                 # [BPC, N, D]
        xTc = np.ascontiguousarray(xc.transpose(0, 2, 1))  # [BPC, D, N]
        in_maps.append({"xT": xTc, **shared})
    return in_maps


def run(x, Wq, bq, Wk, bk, Wv, bv, Wo, trace=False):
    has_bias = bool(np.any(np.asarray(bq)) or np.any(np.asarray(bk))
                    or np.any(np.asarray(bv)))
    nc = _get_nc(has_bias)
    in_maps = make_in_maps(x, Wq, bq, Wk, bk, Wv, bv, Wo)
    res = run_bass_kernel_spmd(nc, in_maps, list(range(N_CORES)), trace=trace)
    out = np.concatenate([res.results[c]["out"] for c in range(N_CORES)], axis=0)
    return out, res


def kernel(x, Wq, bq, Wk, bk, Wv, bv, Wo):
    out, _ = run(x, Wq, bq, Wk, bk, Wv, bv, Wo, trace=False)
    return out


# revision 18
# speedup vs baseline: 1.0990x; 1.0990x over previous
"""Multi-head attention (B=16, N=1024, D=1024, H=8, dh=128) on 8 trn2 cores.

Strategy: data-parallel over batch (2 batches/core), fp32r matmuls.
Per batch on each core:
  phase 1 (per 2-head group g): Q^T_g, K^T_g (head-transposed: dh on
    partitions) and V_g (natural) via fp32r matmuls from x^T (host-side
    pre-transposed) and streamed weight slices.
  phase 2 (per head, per 512-wide q chunk): S^T = K_h^T.T @ Q_h^T (k on
    partitions), E^T = exp(norm*S^T) on ACT, heads^T += V_h.T @ E^T, and
    R = colsum(E^T) via DVE/Pool pairwise adds, then one all-ones 128x128
    matmul that yields R already broadcast to every partition; 1/R via
    a fast 128-lane reciprocal, applied while copying heads^T to SBUF.
  phase 3: out = (heads_norm) @ Wo in natural layout (+ bv@Wo row via a
    K=1 matmul when biases are nonzero).

Scheduling: Wo is resident in SBUF (loaded once), startup DMAs are
ordered so the first projection matmuls start as early as possible,
attention units are queued q-chunk-major so the final batch's output
projection can interleave with the attention drain, and PSUM->SBUF
copies run on the otherwise-idle Pool engine.
"""

import numpy as np

import concourse.bass as bass
import concourse.mybir as mybir
import concourse.tile as tile
from concourse import bacc
from concourse.bass_utils import run_bass_kernel_spmd

N_CORES = 8
B = 16
BPC = B // N_CORES      # batches per core
N = 1024                # sequence length
D = 1024                # model dim
H = 8                   # heads
DH = 128                # head dim
P = 128
DB = D // P             # 8 contraction blocks
GH = 2                  # heads per group
G = H // GH             # 4 groups
GW = GH * DH            # 256: e-width per group
NC2 = N // 512          # 2 n-chunks of 512
NORM = 1.0 / np.sqrt(DH)

F32 = mybir.dt.float32
F32R = mybir.dt.float32r


def r(ap):
    return ap


def build_nc(has_bias=True):
    nc = bacc.Bacc()
    xT = nc.declare_dram_parameter("xT", [BPC, D, N], F32R, isOutput=False)
    Wq = nc.declare_dram_parameter("Wq", [D, D], F32R, isOutput=False)
    Wk = nc.declare_dram_parameter("Wk", [D, D], F32R, isOutput=False)
    Wv = nc.declare_dram_parameter("Wv", [D, D], F32R, isOutput=False)
    Wo = nc.declare_dram_parameter("Wo", [D, D], F32R, isOutput=False)
    bq = nc.declare_dram_parameter("bq", [D], F32, isOutput=False)
    bk = nc.declare_dram_parameter("bk", [D], F32, isOutput=False)
    bv = nc.declare_dram_parameter("bv", [D], F32R, isOutput=False)
    out = nc.declare_dram_parameter("out", [BPC, N, D], F32, isOutput=True)

    ws = [Wq, Wk, Wv]

    with tile.TileContext(nc) as tc:
        with tc.tile_pool(name="big", bufs=1) as big, \
             tc.tile_pool(name="wp", bufs=1) as wp, \
             tc.tile_pool(name="work", bufs=1) as work, \
             tc.tile_pool(name="small", bufs=1) as small, \
             tc.tile_pool(name="ps", bufs=1, space="PSUM") as ps:

            # constants / biases (tiles now; DMAs/memsets emitted after the
            # startup-critical wq/xT DMAs so those win the queue)
            bq_col = small.tile([P, DB], F32, name="bq_col")
            bk_col = small.tile([P, DB], F32, name="bk_col")
            bv_col = small.tile([P, DB], F32R, name="bv_col")
            ones128_f32 = small.tile([P, P], F32, name="ones128_f32")
            ones128 = small.tile([P, P], F32R, name="ones128")
            if has_bias:
                ones_row_f32 = small.tile([1, P], F32, name="ones_row_f32")
                ones_row = small.tile([1, P], F32R, name="ones_row")
                c_sb = small.tile([1, NC2, 512], F32R, name="c_sb")

            def emit_consts():
                nc.sync.dma_start(out=bq_col, in_=bq.rearrange("(eb p) -> p eb", p=P))
                nc.sync.dma_start(out=bk_col, in_=bk.rearrange("(eb p) -> p eb", p=P))
                nc.sync.dma_start(out=bv_col, in_=bv.rearrange("(eb p) -> p eb", p=P))
                nc.vector.memset(ones128_f32, 1.0)
                nc.vector.tensor_copy(ones128, ones128_f32)
                if has_bias:
                    nc.vector.memset(ones_row_f32, 1.0)
                    nc.vector.tensor_copy(ones_row, ones_row_f32)

            # persistent Wo: [128, eb, o] loaded once, shared by both batches
            wo_sb = wp.tile([P, DB, N], F32R, name="wo_sb", tag="wo_sb")
            wo_src = Wo.rearrange("(eb p) o -> p eb o", p=P)

            def issue_wo_dma():
                for q in range(4):
                    nc.sync.dma_start(out=wo_sb[:, 2 * q:2 * q + 2, :],
                                      in_=wo_src[:, 2 * q:2 * q + 2, :])

            def alloc_wgt(b, g):
                gsfx = f"_b{b}_g{g}"
                wgt = {}
                for wname in ("wq", "wk", "wv"):
                    wgt[wname] = wp.tile([P, DB, GW], F32R, name=f"{wname}{gsfx}",
                                         tag="wg", bufs=2)
                return wgt

            def issue_wgt_dma(wgt, g, names=("wq", "wk", "wv")):
                e0 = g * GW
                for wi, wname in enumerate(("wq", "wk", "wv")):
                    if wname not in names:
                        continue
                    wt = wgt[wname]
                    src = ws[wi].rearrange("(db p) e -> p db e", p=P)
                    nc.sync.dma_start(out=wt[:, 0:DB // 2, :],
                                      in_=src[:, 0:DB // 2, e0:e0 + GW])
                    nc.sync.dma_start(out=wt[:, DB // 2:, :],
                                      in_=src[:, DB // 2:, e0:e0 + GW])

            def emit_q_pair(b, g, i0, i1, xt, wgt, qTg):
                """Two Q units with db-interleaved matmuls: during the
                DMA-gated startup the PE gets two matmuls per arriving
                xt d-block instead of one."""
                gsfx = f"_b{b}_g{g}"
                wt = wgt["wq"]
                e0 = g * GW
                pair = []
                for i in (i0, i1):
                    eb, nch = divmod(i, NC2)
                    acc = ps.tile([P, 512], F32, tag="pj", bufs=2,
                                  name=f"pq{gsfx}_{eb}_{nch}")
                    pair.append((eb, nch, acc))
                for db in range(DB):
                    for eb, nch, acc in pair:
                        nc.tensor.matmul(
                            acc,
                            r(wt[:, db, eb * P:(eb + 1) * P]),
                            r(xt[:, db, nch * 512:(nch + 1) * 512]),
                            start=(db == 0), stop=(db == DB - 1))
                for eb, nch, acc in pair:
                    ebg = (e0 // P) + eb
                    if has_bias:
                        nc.vector.tensor_scalar_add(
                            qTg[:, eb, nch * 512:(nch + 1) * 512],
                            acc, bq_col[:, ebg:ebg + 1])
                    else:
                        nc.vector.tensor_copy(
                            qTg[:, eb, nch * 512:(nch + 1) * 512], acc)

            def emit_proj_unit(b, g, kind, idx, xt, wgt, qTg, kTg, vg):
                """Emit one psum accumulation group of phase 1."""
                gsfx = f"_b{b}_g{g}"
                e0 = g * GW
                if kind in ("q", "k"):
                    dst, wt, bcol = ((qTg, wgt["wq"], bq_col) if kind == "q"
                                     else (kTg, wgt["wk"], bk_col))
                    eb, nch = divmod(idx, NC2)
                    acc = ps.tile([P, 512], F32, tag="pj", bufs=2,
                                  name=f"p{kind}{gsfx}_{eb}_{nch}")
                    for db in range(DB):
                        nc.tensor.matmul(
                            acc,
                            r(wt[:, db, eb * P:(eb + 1) * P]),
                            r(xt[:, db, nch * 512:(nch + 1) * 512]),
                            start=(db == 0), stop=(db == DB - 1))
                    ebg = (e0 // P) + eb
                    if has_bias:
                        nc.vector.tensor_scalar_add(
                            dst[:, eb, nch * 512:(nch + 1) * 512],
                            acc, bcol[:, ebg:ebg + 1])
                    else:
                        nc.vector.tensor_copy(
                            dst[:, eb, nch * 512:(nch + 1) * 512], acc)
                else:  # "v"
                    nb = idx
                    accv = ps.tile([P, 512], F32, tag="pj", bufs=2,
                                   name=f"pv{gsfx}_{nb}")
                    for db in range(DB):
                        nc.tensor.matmul(
                            accv[:, :GW],
                            r(xt[:, db, nb * P:(nb + 1) * P]),
                            r(wgt["wv"][:, db, :]),
                            start=(db == 0), stop=(db == DB - 1))
                    nc.vector.tensor_copy(vg[:, nb, :], accv[:, :GW])

            def make_phase3(b, hT):
                """Return list of emit closures: [c-prelude?] + 16 po units
                ordered so the first 8 only need q rows < 512 (qc=0)."""
                units = []
                sfx = f"_b{b}"

                if b == 0 and has_bias:
                    def emit_c():
                        for oc in range(NC2):
                            pc = ps.tile([1, 512], F32, tag="pj", bufs=2,
                                         name=f"pc_{oc}")
                            for eb in range(DB):
                                nc.tensor.matmul(pc, r(bv_col[:, eb:eb + 1]),
                                                 r(wo_sb[:, eb, oc * 512:(oc + 1) * 512]),
                                                 start=(eb == 0), stop=(eb == DB - 1))
                            nc.vector.tensor_copy(c_sb[:, oc, :], pc)
                    units.append(emit_c)

                def make_po(oc, nb):
                    def emit():
                        po = ps.tile([P, 512], F32, tag="pj", bufs=2,
                                     name=f"po{sfx}_{oc}_{nb}")
                        for eb in range(H):
                            nc.tensor.matmul(
                                po,
                                r(hT[:, eb, nb * P:(nb + 1) * P]),
                                r(wo_sb[:, eb, oc * 512:(oc + 1) * 512]),
                                start=(eb == 0),
                                stop=(not has_bias and eb == H - 1))
                        if has_bias:
                            nc.tensor.matmul(po, r(ones_row), r(c_sb[:, oc, :]),
                                             start=False, stop=True)
                        osb = work.tile([P, 512], F32, name=f"o{sfx}_{oc}_{nb}",
                                        tag="osb", bufs=2)
                        nc.scalar.activation(osb, po,
                                             mybir.ActivationFunctionType.Copy)
                        nc.sync.dma_start(
                            out=out[b, nb * P:(nb + 1) * P, oc * 512:(oc + 1) * 512],
                            in_=osb)
                    return emit

                for nb in range(DB // 2):       # q rows < 512 only
                    for oc in range(NC2):
                        units.append(make_po(oc, nb))
                for nb in range(DB // 2, DB):   # q rows >= 512
                    for oc in range(NC2):
                        units.append(make_po(oc, nb))
                return units

            def make_attn(g, hh, qc, qTg, kTg, vg, hT, b):
                """Split attention unit: (emit_scores, emit_av)."""
                h = g * GH + hh
                asfx = f"_b{b}_h{h}_q{qc}"
                st = {}
                add = mybir.AluOpType.add

                def emit_scores():
                    eT = work.tile([P, 4, 1024], F32R, name=f"eT{asfx}",
                                   tag="eT", bufs=(1 if has_bias else 2))
                    st["eT"] = eT
                    for j in range(4):
                        # scores for kb=2j, 2j+1 into one 2-bank tile
                        sp = ps.tile([P, 1024], F32, tag="spair", bufs=2,
                                     name=f"sp{asfx}_{j}")
                        for half in range(2):
                            kb = 2 * j + half
                            nc.tensor.matmul(
                                sp[:, half * 512:(half + 1) * 512],
                                r(kTg[:, hh, kb * P:(kb + 1) * P]),
                                r(qTg[:, hh, qc * 512:(qc + 1) * 512]),
                                start=True, stop=True)
                        nc.scalar.activation(
                            eT[:, j, :], sp,
                            mybir.ActivationFunctionType.Exp,
                            scale=float(NORM))

                def emit_av():
                    eT = st["eT"]
                    # heads^T (unnormalized): [dv(128) x q(512)]
                    pav = ps.tile([P, 512], F32, tag="pav", bufs=1,
                                  name=f"pav{asfx}")
                    for j in range(4):
                        for half in range(2):
                            kb = 2 * j + half
                            nc.tensor.matmul(
                                pav,
                                r(vg[:, kb, hh * DH:(hh + 1) * DH]),
                                r(eT[:, j, half * 512:(half + 1) * 512]),
                                start=(kb == 0), stop=(kb == DB - 1))

                    # R = col-sum of E^T: serial accumulate on DVE (keeps the
                    # post-exp dependency chain short: last exp -> 2 adds)
                    tA = work.tile([P, 1024], F32R, name=f"tA{asfx}", tag="tA", bufs=1)
                    rp = work.tile([P, 512], F32R, name=f"rp{asfx}", tag="rp", bufs=1)
                    nc.vector.tensor_tensor(tA, eT[:, 0, :], eT[:, 1, :], add)
                    nc.vector.tensor_tensor(tA, tA, eT[:, 2, :], add)
                    nc.vector.tensor_tensor(tA, tA, eT[:, 3, :], add)
                    nc.vector.tensor_tensor(rp, tA[:, 0:512], tA[:, 512:1024], add)
                    # colsum of rp, broadcast to all partitions, in
                    # one matmul: every row of ones128.T @ rp is R
                    pbc = ps.tile([P, 512], F32, tag="pnorm", bufs=1, name=f"pbc{asfx}")
                    nc.tensor.matmul(pbc, r(ones128), r(rp),
                                     start=True, stop=True)
                    # 1/R at full 128-lane width (approx + one NR pass)
                    # (scratch shares the rp slot: rp's only reader, the pbc
                    # matmul, always precedes the reciprocal that writes it)
                    scratch = work.tile([P, 512], F32, name=f"sc{asfx}",
                                        tag="rp", bufs=1)
                    binv = work.tile([P, 512], F32, name=f"binv{asfx}",
                                     tag="binv", bufs=1)
                    nc.vector.reciprocal_approx_accurate(binv, pbc, scratch)
                    nc.vector.tensor_tensor(
                        hT[:, h, qc * 512:(qc + 1) * 512], pav, binv,
                        mybir.AluOpType.mult)

                return emit_scores, emit_av

            # attention units and the previous batch's output projection are
            # emitted interleaved with later projection units so PE always
            # has ready matmuls during exp/epilogue waits
            attn_queue = []
            pending_phase3 = None

            for b in range(BPC):
                sfx = f"_b{b}"
                xt = big.tile([P, DB, N], F32R, name=f"xt{sfx}", tag="xt")
                xsrc = xT[b].rearrange("(db p) n -> p db n", p=P)

                wgt0 = None
                if b == 0:
                    # startup: wq for group 0 must land before anything else
                    # so the first projection matmuls can begin immediately
                    wgt0 = alloc_wgt(0, 0)
                    issue_wgt_dma(wgt0, 0, names=("wq",))
                for db in range(DB):
                    nc.sync.dma_start(out=xt[:, db, :], in_=xsrc[:, db, :])
                if b == 0:
                    issue_wgt_dma(wgt0, 0, names=("wk", "wv"))
                    emit_consts()

                hT = None

                for g in range(G):
                    # ---- weight slices for this group: [128, db, GW]
                    if b == 0 and g == 0:
                        wgt = wgt0
                    else:
                        wgt = alloc_wgt(b, g)
                        issue_wgt_dma(wgt, g)
                    if b == 0 and g == 1:
                        issue_wo_dma()

                    qTg = work.tile([P, GH, N], F32R, name=f"qT{sfx}_g{g}", tag="qTg", bufs=2)
                    kTg = work.tile([P, GH, N], F32R, name=f"kT{sfx}_g{g}", tag="kTg", bufs=2)
                    vg = work.tile([P, DB, GW], F32R, name=f"v{sfx}_g{g}", tag="vg", bufs=2)

                    # 16 proj units: 4 Q, 4 K, 8 V; interleave with up to 4
                    # pending attention units (1 attention per 4 proj units)
                    if b == 0 and g == 0:
                        emit_q_pair(b, g, 0, 1, xt, wgt, qTg)
                        emit_q_pair(b, g, 2, 3, xt, wgt, qTg)
                        units = ([("k", i) for i in range(GH * NC2)]
                                 + [("v", i) for i in range(DB)])
                    else:
                        units = ([("q", i) for i in range(GH * NC2)]
                                 + [("k", i) for i in range(GH * NC2)]
                                 + [("v", i) for i in range(DB)])
                    for ui, (kind, idx) in enumerate(units):
                        emit_proj_unit(b, g, kind, idx, xt, wgt, qTg, kTg, vg)
                        if ui % 4 == 1 and attn_queue:
                            s_fn, a_fn = attn_queue.pop(0)
                            s_fn(); a_fn()
                    while attn_queue:
                        s_fn, a_fn = attn_queue.pop(0)
                        s_fn(); a_fn()
                    if pending_phase3 is not None:
                        for u in pending_phase3:
                            u()
                        pending_phase3 = None
                    if hT is None:
                        hT = big.tile([P, H, N], F32R, name=f"hT{sfx}", tag="hT")

                    # ---- queue attention for the heads of this group,
                    # q-chunk-major so all heads' qc=0 results land first
                    for qc in range(NC2):
                        for hh in range(GH):
                            attn_queue.append(
                                make_attn(g, hh, qc, qTg, kTg, vg, hT, b))

                if b < BPC - 1:
                    # phase 3 of this batch is deferred: it is emitted after
                    # the next batch's first projection group so its matmuls
                    # overlap the last attention units
                    pending_phase3 = make_phase3(b, hT)
                else:
                    # final batch: interleave the last group's attention with
                    # the output projection so PE keeps working through the
                    # exp chains of the final units
                    p3 = make_phase3(b, hT)
                    pre = p3[:-16]        # c-prelude if present (b==0 case)
                    po = p3[-16:]         # po[:8] need qc=0 only
                    drain = list(attn_queue)
                    attn_queue = []
                    # qc=0 units (first half of queue): run normally
                    for s_fn, a_fn in drain[:-2]:
                        s_fn(); a_fn()
                    for u in pre:
                        u()
                    # last two (qc=1) units: fill exp latency with po units
                    (s0, a0), (s1, a1) = drain[-2:]
                    s0()
                    po[0](); po[1]()
                    a0()
                    s1()
                    po[2](); po[3]()
                    a1()
                    for u in po[4:]:
                        u()

            # tail: drain remaining attention, then the last output projection
            while attn_queue:
                s_fn, a_fn = attn_queue.pop(0)
                s_fn(); a_fn()
            if pending_phase3 is not None:
                for u in pending_phase3:
                    u()

    nc.compile()
    return nc


_NC_CACHE = {}


def _get_nc(has_bias):
    if has_bias not in _NC_CACHE:
        _NC_CACHE[has_bias] = build_nc(has_bias)
    return _NC_CACHE[has_bias]


def make_in_maps(x, Wq, bq, Wk, bk, Wv, bv, Wo):
    x = np.asarray(x, dtype=np.float32)
    in_maps = []
    shared = {
        "Wq": np.ascontiguousarray(Wq, dtype=np.float32),
        "Wk": np.ascontiguousarray(Wk, dtype=np.float32),
        "Wv": np.ascontiguousarray(Wv, dtype=np.float32),
        "Wo": np.ascontiguousarray(Wo, dtype=np.float32),
        "bq": np.ascontiguousarray(bq, dtype=np.float32),
        "bk": np.ascontiguousarray(bk, dtype=np.float32),
        "bv": np.ascontiguousarray(bv, dtype=np.float32),
    }
    for c in range(N_CORES):
        xc = x[c * BPC:(c + 1) * BPC]                 # [BPC, N, D]
        xTc = np.ascontiguousarray(xc.transpose(0, 2, 1))  # [BPC, D, N]
        in_maps.append({"xT": xTc, **shared})
    return in_maps


def run(x, Wq, bq, Wk, bk, Wv, bv, Wo, trace=False):
    has_bias = bool(np.any(np.asarray(bq)) or np.any(np.asarray(bk))
                    or np.any(np.asarray(bv)))
    nc = _get_nc(has_bias)
    in_maps = make_in_maps(x, Wq, bq, Wk, bk, Wv, bv, Wo)
    res = run_bass_kernel_spmd(nc, in_maps, list(range(N_CORES)), trace=trace)
    out = np.concatenate([res.results[c]["out"] for c in range(N_CORES)], axis=0)
    return out, res


def kernel(x, Wq, bq, Wk, bk, Wv, bv, Wo):
    out, _ = run(x, Wq, bq, Wk, bk, Wv, bv, Wo, trace=False)
    return out


# revision 24
# speedup vs baseline: 1.1525x; 1.0486x over previous
"""Multi-head attention (B=16, N=1024, D=1024, H=8, dh=128) on 8 trn2 cores.

Strategy: data-parallel over batch (2 batches/core), fp32r matmuls.
Per batch on each core:
  phase 1 (per 2-head group g): Q^T_g, K^T_g (head-transposed: dh on
    partitions) and V_g (natural) via fp32r matmuls from x^T (host-side
    pre-transposed) and streamed weight slices.
  phase 2 (per head, per 512-wide q chunk): S^T = K_h^T.T @ Q_h^T (k on
    partitions), E^T = exp(norm*S^T) on ACT, heads^T += V_h.T @ E^T, and
    R = colsum(E^T) via DVE/Pool pairwise adds, then one all-ones 128x128
    matmul that yields R already broadcast to every partition; 1/R via
    a fast 128-lane reciprocal, applied while copying heads^T to SBUF.
  phase 3: out = (heads_norm) @ Wo in natural layout (+ bv@Wo row via a
    K=1 matmul when biases are nonzero).

Scheduling: Wo is resident in SBUF (loaded once), startup DMAs are
ordered so the first projection matmuls start as early as possible,
attention units are queued q-chunk-major so the final batch's output
projection can interleave with the attention drain, and PSUM->SBUF
copies run on the otherwise-idle Pool engine.
"""

import numpy as np

import concourse.bass as bass
import concourse.mybir as mybir
import concourse.tile as tile
from concourse import bacc
from concourse.bass_utils import run_bass_kernel_spmd

N_CORES = 8
B = 16
BPC = B // N_CORES      # batches per core
N = 1024                # sequence length
D = 1024                # model dim
H = 8                   # heads
DH = 128                # head dim
P = 128
DB = D // P             # 8 contraction blocks
GH = 2                  # heads per group
G = H // GH             # 4 groups
GW = GH * DH            # 256: e-width per group
NC2 = N // 512          # 2 n-chunks of 512
NORM = 1.0 / np.sqrt(DH)

F32 = mybir.dt.float32
F32R = mybir.dt.float32r
BF16 = mybir.dt.bfloat16


def r(ap):
    return ap


def build_nc(has_bias=True):
    nc = bacc.Bacc()
    xT = nc.declare_dram_parameter("xT", [BPC, D, N], BF16, isOutput=False)
    Wq = nc.declare_dram_parameter("Wq", [D, D], BF16, isOutput=False)
    Wk = nc.declare_dram_parameter("Wk", [D, D], BF16, isOutput=False)
    Wv = nc.declare_dram_parameter("Wv", [D, D], BF16, isOutput=False)
    Wo = nc.declare_dram_parameter("Wo", [D, D], F32R, isOutput=False)
    bq = nc.declare_dram_parameter("bq", [D], F32, isOutput=False)
    bk = nc.declare_dram_parameter("bk", [D], F32, isOutput=False)
    bv = nc.declare_dram_parameter("bv", [D], F32R, isOutput=False)
    out = nc.declare_dram_parameter("out", [BPC, N, D], F32, isOutput=True)

    ws = [Wq, Wk, Wv]

    with tile.TileContext(nc) as tc:
        with tc.tile_pool(name="big", bufs=1) as big, \
             tc.tile_pool(name="wp", bufs=1) as wp, \
             tc.tile_pool(name="work", bufs=1) as work, \
             tc.tile_pool(name="small", bufs=1) as small, \
             tc.tile_pool(name="ps", bufs=1, space="PSUM") as ps:

            # constants / biases (tiles now; DMAs/memsets emitted after the
            # startup-critical wq/xT DMAs so those win the queue)
            bq_col = small.tile([P, DB], F32, name="bq_col")
            bk_col = small.tile([P, DB], F32, name="bk_col")
            bv_col = small.tile([P, DB], F32R, name="bv_col")
            ones128_f32 = small.tile([P, P], F32, name="ones128_f32")
            ones128 = small.tile([P, P], F32R, name="ones128")
            if has_bias:
                ones_row_f32 = small.tile([1, P], F32, name="ones_row_f32")
                ones_row = small.tile([1, P], F32R, name="ones_row")
                c_sb = small.tile([1, NC2, 512], F32R, name="c_sb")

            def emit_consts():
                if has_bias:
                    nc.sync.dma_start(out=bq_col, in_=bq.rearrange("(eb p) -> p eb", p=P))
                    nc.sync.dma_start(out=bk_col, in_=bk.rearrange("(eb p) -> p eb", p=P))
                    nc.sync.dma_start(out=bv_col, in_=bv.rearrange("(eb p) -> p eb", p=P))
                nc.vector.memset(ones128_f32, 1.0)
                nc.vector.tensor_copy(ones128, ones128_f32)
                if has_bias:
                    nc.vector.memset(ones_row_f32, 1.0)
                    nc.vector.tensor_copy(ones_row, ones_row_f32)

            # persistent Wo: [128, eb, o] loaded once, shared by both batches
            wo_sb = wp.tile([P, DB, N], F32R, name="wo_sb", tag="wo_sb")
            wo_src = Wo.rearrange("(eb p) o -> p eb o", p=P)

            def issue_wo_dma():
                for q in range(4):
                    nc.sync.dma_start(out=wo_sb[:, 2 * q:2 * q + 2, :],
                                      in_=wo_src[:, 2 * q:2 * q + 2, :])

            def alloc_wgt(b, g):
                gsfx = f"_b{b}_g{g}"
                wgt = {}
                for wname in ("wq", "wk", "wv"):
                    wgt[wname] = wp.tile([P, DB, GW], BF16, name=f"{wname}{gsfx}",
                                         tag="wg", bufs=2)
                return wgt

            def issue_wgt_dma(wgt, g, names=("wq", "wk", "wv")):
                e0 = g * GW
                for wi, wname in enumerate(("wq", "wk", "wv")):
                    if wname not in names:
                        continue
                    wt = wgt[wname]
                    src = ws[wi].rearrange("(db p) e -> p db e", p=P)
                    nc.sync.dma_start(out=wt[:, 0:DB // 2, :],
                                      in_=src[:, 0:DB // 2, e0:e0 + GW])
                    nc.sync.dma_start(out=wt[:, DB // 2:, :],
                                      in_=src[:, DB // 2:, e0:e0 + GW])

            def emit_q_pair(b, g, i0, i1, xt, wgt, qTg):
                """Two Q units with db-interleaved matmuls: during the
                DMA-gated startup the PE gets two matmuls per arriving
                xt d-block instead of one."""
                gsfx = f"_b{b}_g{g}"
                wt = wgt["wq"]
                e0 = g * GW
                pair = []
                for i in (i0, i1):
                    eb, nch = divmod(i, NC2)
                    acc = ps.tile([P, 512], F32, tag="pj", bufs=2,
                                  name=f"pq{gsfx}_{eb}_{nch}")
                    pair.append((eb, nch, acc))
                for db in range(DB):
                    for eb, nch, acc in pair:
                        nc.tensor.matmul(
                            acc,
                            r(wt[:, db, eb * P:(eb + 1) * P]),
                            r(xt[:, db, nch * 512:(nch + 1) * 512]),
                            start=(db == 0), stop=(db == DB - 1))
                for eb, nch, acc in pair:
                    ebg = (e0 // P) + eb
                    if has_bias:
                        nc.vector.tensor_scalar_add(
                            qTg[:, eb, nch * 512:(nch + 1) * 512],
                            acc, bq_col[:, ebg:ebg + 1])
                    else:
                        nc.vector.tensor_copy(
                            qTg[:, eb, nch * 512:(nch + 1) * 512], acc)

            def emit_proj_unit(b, g, kind, idx, xt, wgt, qTg, kTg, vg):
                """Emit one psum accumulation group of phase 1."""
                gsfx = f"_b{b}_g{g}"
                e0 = g * GW
                if kind in ("q", "k"):
                    dst, wt, bcol = ((qTg, wgt["wq"], bq_col) if kind == "q"
                                     else (kTg, wgt["wk"], bk_col))
                    eb, nch = divmod(idx, NC2)
                    acc = ps.tile([P, 512], F32, tag="pj", bufs=2,
                                  name=f"p{kind}{gsfx}_{eb}_{nch}")
                    for db in range(DB):
                        nc.tensor.matmul(
                            acc,
                            r(wt[:, db, eb * P:(eb + 1) * P]),
                            r(xt[:, db, nch * 512:(nch + 1) * 512]),
                            start=(db == 0), stop=(db == DB - 1))
                    ebg = (e0 // P) + eb
                    if has_bias:
                        nc.vector.tensor_scalar_add(
                            dst[:, eb, nch * 512:(nch + 1) * 512],
                            acc, bcol[:, ebg:ebg + 1])
                    else:
                        nc.vector.tensor_copy(
                            dst[:, eb, nch * 512:(nch + 1) * 512], acc)
                else:  # "v"
                    nb = idx
                    accv = ps.tile([P, 512], F32, tag="pj", bufs=2,
                                   name=f"pv{gsfx}_{nb}")
                    for db in range(DB):
                        nc.tensor.matmul(
                            accv[:, :GW],
                            r(xt[:, db, nb * P:(nb + 1) * P]),
                            r(wgt["wv"][:, db, :]),
                            start=(db == 0), stop=(db == DB - 1))
                    nc.vector.tensor_copy(vg[:, nb, :], accv[:, :GW])

            def make_phase3(b, hT):
                """Return list of emit closures: [c-prelude?] + 16 po units
                ordered so the first 8 only need q rows < 512 (qc=0)."""
                units = []
                sfx = f"_b{b}"

                if b == 0 and has_bias:
                    def emit_c():
                        for oc in range(NC2):
                            pc = ps.tile([1, 512], F32, tag="pj", bufs=2,
                                         name=f"pc_{oc}")
                            for eb in range(DB):
                                nc.tensor.matmul(pc, r(bv_col[:, eb:eb + 1]),
                                                 r(wo_sb[:, eb, oc * 512:(oc + 1) * 512]),
                                                 start=(eb == 0), stop=(eb == DB - 1))
                            nc.vector.tensor_copy(c_sb[:, oc, :], pc)
                    units.append(emit_c)

                def make_po(oc, nb):
                    def emit():
                        po = ps.tile([P, 512], F32, tag="pj", bufs=2,
                                     name=f"po{sfx}_{oc}_{nb}")
                        for eb in range(H):
                            nc.tensor.matmul(
                                po,
                                r(hT[:, eb, nb * P:(nb + 1) * P]),
                                r(wo_sb[:, eb, oc * 512:(oc + 1) * 512]),
                                start=(eb == 0),
                                stop=(not has_bias and eb == H - 1))
                        if has_bias:
                            nc.tensor.matmul(po, r(ones_row), r(c_sb[:, oc, :]),
                                             start=False, stop=True)
                        osb = work.tile([P, 512], F32, name=f"o{sfx}_{oc}_{nb}",
                                        tag="osb", bufs=2)
                        nc.scalar.activation(osb, po,
                                             mybir.ActivationFunctionType.Copy)
                        nc.sync.dma_start(
                            out=out[b, nb * P:(nb + 1) * P, oc * 512:(oc + 1) * 512],
                            in_=osb)
                    return emit

                for nb in range(DB // 2):       # q rows < 512 only
                    for oc in range(NC2):
                        units.append(make_po(oc, nb))
                for nb in range(DB // 2, DB):   # q rows >= 512
                    for oc in range(NC2):
                        units.append(make_po(oc, nb))
                return units

            def make_attn(g, hh, qc, qTg, kTg, vg, hT, b):
                """Split attention unit: (emit_scores, emit_av)."""
                h = g * GH + hh
                asfx = f"_b{b}_h{h}_q{qc}"
                st = {}
                add = mybir.AluOpType.add

                def emit_scores():
                    eT = work.tile([P, 4, 1024], F32R, name=f"eT{asfx}",
                                   tag="eT", bufs=(1 if has_bias else 2))
                    st["eT"] = eT
                    for j in range(4):
                        # scores for kb=2j, 2j+1 into one 2-bank tile
                        sp = ps.tile([P, 1024], F32, tag="spair", bufs=2,
                                     name=f"sp{asfx}_{j}")
                        for half in range(2):
                            kb = 2 * j + half
                            nc.tensor.matmul(
                                sp[:, half * 512:(half + 1) * 512],
                                r(kTg[:, hh, kb * P:(kb + 1) * P]),
                                r(qTg[:, hh, qc * 512:(qc + 1) * 512]),
                                start=True, stop=True)
                        nc.scalar.activation(
                            eT[:, j, :], sp,
                            mybir.ActivationFunctionType.Exp,
                            scale=float(NORM))

                def emit_av():
                    eT = st["eT"]
                    # heads^T (unnormalized): [dv(128) x q(512)]
                    pav = ps.tile([P, 512], F32, tag="pav", bufs=1,
                                  name=f"pav{asfx}")
                    for j in range(4):
                        for half in range(2):
                            kb = 2 * j + half
                            nc.tensor.matmul(
                                pav,
                                r(vg[:, kb, hh * DH:(hh + 1) * DH]),
                                r(eT[:, j, half * 512:(half + 1) * 512]),
                                start=(kb == 0), stop=(kb == DB - 1))

                    # R = col-sum of E^T: serial accumulate on DVE (keeps the
                    # post-exp dependency chain short: last exp -> 2 adds)
                    tA = work.tile([P, 1024], F32R, name=f"tA{asfx}", tag="tA", bufs=1)
                    rp = work.tile([P, 512], F32R, name=f"rp{asfx}", tag="rp", bufs=1)
                    nc.vector.tensor_tensor(tA, eT[:, 0, :], eT[:, 1, :], add)
                    nc.vector.tensor_tensor(tA, tA, eT[:, 2, :], add)
                    nc.vector.tensor_tensor(tA, tA, eT[:, 3, :], add)
                    nc.vector.tensor_tensor(rp, tA[:, 0:512], tA[:, 512:1024], add)
                    # colsum of rp, broadcast to all partitions, in
                    # one matmul: every row of ones128.T @ rp is R
                    pbc = ps.tile([P, 512], F32, tag="pnorm", bufs=1, name=f"pbc{asfx}")
                    nc.tensor.matmul(pbc, r(ones128), r(rp),
                                     start=True, stop=True)
                    # 1/R at full 128-lane width (approx + one NR pass)
                    # (scratch shares the rp slot: rp's only reader, the pbc
                    # matmul, always precedes the reciprocal that writes it)
                    scratch = work.tile([P, 512], F32, name=f"sc{asfx}",
                                        tag="rp", bufs=1)
                    binv = work.tile([P, 512], F32, name=f"binv{asfx}",
                                     tag="binv", bufs=1)
                    nc.vector.reciprocal_approx_accurate(binv, pbc, scratch)
                    nc.vector.tensor_tensor(
                        hT[:, h, qc * 512:(qc + 1) * 512], pav, binv,
                        mybir.AluOpType.mult)

                return emit_scores, emit_av

            # attention units and the previous batch's output projection are
            # emitted interleaved with later projection units so PE always
            # has ready matmuls during exp/epilogue waits
            attn_queue = []
            pending_phase3 = None

            xt_tiles = {}

            def load_xt(b):
                xtl = big.tile([P, DB, N], BF16, name=f"xt_b{b}", tag="xt",
                               bufs=2)
                xsrc = xT[b].rearrange("(db p) n -> p db n", p=P)
                for db in range(DB):
                    nc.sync.dma_start(out=xtl[:, db, :], in_=xsrc[:, db, :])
                xt_tiles[b] = xtl

            for b in range(BPC):
                sfx = f"_b{b}"

                wgt0 = None
                if b == 0:
                    # startup: wq for group 0 must land before anything else
                    # so the first projection matmuls can begin immediately
                    wgt0 = alloc_wgt(0, 0)
                    issue_wgt_dma(wgt0, 0, names=("wq",))
                    load_xt(0)
                    issue_wgt_dma(wgt0, 0, names=("wk", "wv"))
                    emit_consts()
                xt = xt_tiles[b]

                hT = None

                for g in range(G):
                    # ---- weight slices for this group: [128, db, GW]
                    if b == 0 and g == 0:
                        wgt = wgt0
                    else:
                        wgt = alloc_wgt(b, g)
                        issue_wgt_dma(wgt, g)
                    if b == 0 and g == 1:
                        issue_wo_dma()
                    if g == G - 1 and b + 1 < BPC:
                        # prefetch the next batch's x^T while this batch's
                        # last attention/projection work is still running
                        load_xt(b + 1)

                    qTg = work.tile([P, GH, N], F32R, name=f"qT{sfx}_g{g}", tag="qTg", bufs=2)
                    kTg = work.tile([P, GH, N], F32R, name=f"kT{sfx}_g{g}", tag="kTg", bufs=2)
                    vg = work.tile([P, DB, GW], F32R, name=f"v{sfx}_g{g}", tag="vg", bufs=2)

                    # 16 proj units: 4 Q, 4 K, 8 V; interleave with up to 4
                    # pending attention units (1 attention per 4 proj units)
                    if b == 0 and g == 0:
                        emit_q_pair(b, g, 0, 1, xt, wgt, qTg)
                        emit_q_pair(b, g, 2, 3, xt, wgt, qTg)
                        units = ([("k", i) for i in range(GH * NC2)]
                                 + [("v", i) for i in range(DB)])
                    else:
                        units = ([("q", i) for i in range(GH * NC2)]
                                 + [("k", i) for i in range(GH * NC2)]
                                 + [("v", i) for i in range(DB)])
                    for ui, (kind, idx) in enumerate(units):
                        emit_proj_unit(b, g, kind, idx, xt, wgt, qTg, kTg, vg)
                        if ui % 4 == 1 and attn_queue:
                            s_fn, a_fn = attn_queue.pop(0)
                            s_fn(); a_fn()
                    while attn_queue:
                        s_fn, a_fn = attn_queue.pop(0)
                        s_fn(); a_fn()
                    if pending_phase3 is not None:
                        for u in pending_phase3:
                            u()
                        pending_phase3 = None
                    if hT is None:
                        hT = big.tile([P, H, N], F32R, name=f"hT{sfx}", tag="hT")

                    # ---- queue attention for the heads of this group,
                    # q-chunk-major so all heads' qc=0 results land first
                    for qc in range(NC2):
                        for hh in range(GH):
                            attn_queue.append(
                                make_attn(g, hh, qc, qTg, kTg, vg, hT, b))

                if b < BPC - 1:
                    # phase 3 of this batch is deferred: it is emitted after
                    # the next batch's first projection group so its matmuls
                    # overlap the last attention units
                    pending_phase3 = make_phase3(b, hT)
                else:
                    # final batch: interleave the last group's attention with
                    # the output projection so PE keeps working through the
                    # exp chains of the final units
                    p3 = make_phase3(b, hT)
                    pre = p3[:-16]        # c-prelude if present (b==0 case)
                    po = p3[-16:]         # po[:8] need qc=0 only
                    drain = list(attn_queue)
                    attn_queue = []
                    # qc=0 units (first half of queue): run normally
                    for s_fn, a_fn in drain[:-2]:
                        s_fn(); a_fn()
                    for u in pre:
                        u()
                    # last two (qc=1) units: fill exp latency with po units
                    (s0, a0), (s1, a1) = drain[-2:]
                    s0()
                    po[0](); po[1]()
                    a0()
                    s1()
                    po[2](); po[3]()
                    a1()
                    for u in po[4:]:
                        u()

            # tail: drain remaining attention, then the last output projection
            while attn_queue:
                s_fn, a_fn = attn_queue.pop(0)
                s_fn(); a_fn()
            if pending_phase3 is not None:
                for u in pending_phase3:
                    u()

    nc.compile()
    return nc


_NC_CACHE = {}


def _get_nc(has_bias):
    if has_bias not in _NC_CACHE:
        _NC_CACHE[has_bias] = build_nc(has_bias)
    return _NC_CACHE[has_bias]


def make_in_maps(x, Wq, bq, Wk, bk, Wv, bv, Wo):
    import ml_dtypes
    bf16 = ml_dtypes.bfloat16
    x = np.asarray(x, dtype=np.float32)
    in_maps = []
    shared = {
        "Wq": np.ascontiguousarray(np.asarray(Wq, dtype=np.float32).astype(bf16)),
        "Wk": np.ascontiguousarray(np.asarray(Wk, dtype=np.float32).astype(bf16)),
        "Wv": np.ascontiguousarray(np.asarray(Wv, dtype=np.float32).astype(bf16)),
        "Wo": np.ascontiguousarray(Wo, dtype=np.float32),
        "bq": np.ascontiguousarray(bq, dtype=np.float32),
        "bk": np.ascontiguousarray(bk, dtype=np.float32),
        "bv": np.ascontiguousarray(bv, dtype=np.float32),
    }
    for c in range(N_CORES):
        xc = x[c * BPC:(c + 1) * BPC]                 # [BPC, N, D]
        xTc = np.ascontiguousarray(xc.transpose(0, 2, 1).astype(bf16))
        in_maps.append({"xT": xTc, **shared})
    return in_maps


def run(x, Wq, bq, Wk, bk, Wv, bv, Wo, trace=False):
    has_bias = bool(np.any(np.asarray(bq)) or np.any(np.asarray(bk))
                    or np.any(np.asarray(bv)))
    nc = _get_nc(has_bias)
    in_maps = make_in_maps(x, Wq, bq, Wk, bk, Wv, bv, Wo)
    res = run_bass_kernel_spmd(nc, in_maps, list(range(N_CORES)), trace=trace)
    out = np.concatenate([res.results[c]["out"] for c in range(N_CORES)], axis=0)
    return out, res


def kernel(x, Wq, bq, Wk, bk, Wv, bv, Wo):
    out, _ = run(x, Wq, bq, Wk, bk, Wv, bv, Wo, trace=False)
    return out


# revision 25
# speedup vs baseline: 1.1889x; 1.0316x over previous
"""Multi-head attention (B=16, N=1024, D=1024, H=8, dh=128) on 8 trn2 cores.

Strategy: data-parallel over batch (2 batches/core), fp32r matmuls.
Per batch on each core:
  phase 1 (per 2-head group g): Q^T_g, K^T_g (head-transposed: dh on
    partitions) and V_g (natural) via fp32r matmuls from x^T (host-side
    pre-transposed) and streamed weight slices.
  phase 2 (per head, per 512-wide q chunk): S^T = K_h^T.T @ Q_h^T (k on
    partitions), E^T = exp(norm*S^T) on ACT, heads^T += V_h.T @ E^T, and
    R = colsum(E^T) via DVE/Pool pairwise adds, then one all-ones 128x128
    matmul that yields R already broadcast to every partition; 1/R via
    a fast 128-lane reciprocal, applied while copying heads^T to SBUF.
  phase 3: out = (heads_norm) @ Wo in natural layout (+ bv@Wo row via a
    K=1 matmul when biases are nonzero).

Scheduling: Wo is resident in SBUF (loaded once), startup DMAs are
ordered so the first projection matmuls start as early as possible,
attention units are queued q-chunk-major so the final batch's output
projection can interleave with the attention drain, and PSUM->SBUF
copies run on the otherwise-idle Pool engine.
"""

import numpy as np

import concourse.bass as bass
import concourse.mybir as mybir
import concourse.tile as tile
from concourse import bacc
from concourse.bass_utils import run_bass_kernel_spmd

N_CORES = 8
B = 16
BPC = B // N_CORES      # batches per core
N = 1024                # sequence length
D = 1024                # model dim
H = 8                   # heads
DH = 128                # head dim
P = 128
DB = D // P             # 8 contraction blocks
GH = 2                  # heads per group
G = H // GH             # 4 groups
GW = GH * DH            # 256: e-width per group
NC2 = N // 512          # 2 n-chunks of 512
NORM = 1.0 / np.sqrt(DH)

F32 = mybir.dt.float32
F32R = mybir.dt.float32r
BF16 = mybir.dt.bfloat16


def r(ap):
    return ap


def build_nc(has_bias=True):
    nc = bacc.Bacc()
    xT = nc.declare_dram_parameter("xT", [BPC, D, N], BF16, isOutput=False)
    Wq = nc.declare_dram_parameter("Wq", [D, D], BF16, isOutput=False)
    Wk = nc.declare_dram_parameter("Wk", [D, D], BF16, isOutput=False)
    Wv = nc.declare_dram_parameter("Wv", [D, D], BF16, isOutput=False)
    Wo = nc.declare_dram_parameter("Wo", [D, D], BF16, isOutput=False)
    bq = nc.declare_dram_parameter("bq", [D], F32, isOutput=False)
    bk = nc.declare_dram_parameter("bk", [D], F32, isOutput=False)
    bv = nc.declare_dram_parameter("bv", [D], BF16, isOutput=False)
    out = nc.declare_dram_parameter("out", [BPC, N, D], F32, isOutput=True)

    ws = [Wq, Wk, Wv]

    with tile.TileContext(nc) as tc:
        with tc.tile_pool(name="big", bufs=1) as big, \
             tc.tile_pool(name="wp", bufs=1) as wp, \
             tc.tile_pool(name="work", bufs=1) as work, \
             tc.tile_pool(name="small", bufs=1) as small, \
             tc.tile_pool(name="ps", bufs=1, space="PSUM") as ps:

            # constants / biases (tiles now; DMAs/memsets emitted after the
            # startup-critical wq/xT DMAs so those win the queue)
            bq_col = small.tile([P, DB], F32, name="bq_col")
            bk_col = small.tile([P, DB], F32, name="bk_col")
            bv_col = small.tile([P, DB], BF16, name="bv_col")
            ones128_f32 = small.tile([P, P], F32, name="ones128_f32")
            ones128 = small.tile([P, P], F32R, name="ones128")
            if has_bias:
                ones_row_f32 = small.tile([1, P], F32, name="ones_row_f32")
                ones_row = small.tile([1, P], BF16, name="ones_row")
                c_sb = small.tile([1, NC2, 512], BF16, name="c_sb")

            def emit_consts():
                if has_bias:
                    nc.sync.dma_start(out=bq_col, in_=bq.rearrange("(eb p) -> p eb", p=P))
                    nc.sync.dma_start(out=bk_col, in_=bk.rearrange("(eb p) -> p eb", p=P))
                    nc.sync.dma_start(out=bv_col, in_=bv.rearrange("(eb p) -> p eb", p=P))
                nc.vector.memset(ones128_f32, 1.0)
                nc.vector.tensor_copy(ones128, ones128_f32)
                if has_bias:
                    nc.vector.memset(ones_row_f32, 1.0)
                    nc.vector.tensor_copy(ones_row, ones_row_f32)

            # persistent Wo: [128, eb, o] loaded once, shared by both batches
            wo_sb = wp.tile([P, DB, N], BF16, name="wo_sb", tag="wo_sb")
            wo_src = Wo.rearrange("(eb p) o -> p eb o", p=P)

            def issue_wo_dma():
                for q in range(4):
                    nc.sync.dma_start(out=wo_sb[:, 2 * q:2 * q + 2, :],
                                      in_=wo_src[:, 2 * q:2 * q + 2, :])

            def alloc_wgt(b, g):
                gsfx = f"_b{b}_g{g}"
                wgt = {}
                for wname in ("wq", "wk", "wv"):
                    wgt[wname] = wp.tile([P, DB, GW], BF16, name=f"{wname}{gsfx}",
                                         tag="wg", bufs=2)
                return wgt

            def issue_wgt_dma(wgt, g, names=("wq", "wk", "wv")):
                e0 = g * GW
                for wi, wname in enumerate(("wq", "wk", "wv")):
                    if wname not in names:
                        continue
                    wt = wgt[wname]
                    src = ws[wi].rearrange("(db p) e -> p db e", p=P)
                    nc.sync.dma_start(out=wt[:, 0:DB // 2, :],
                                      in_=src[:, 0:DB // 2, e0:e0 + GW])
                    nc.sync.dma_start(out=wt[:, DB // 2:, :],
                                      in_=src[:, DB // 2:, e0:e0 + GW])

            def emit_q_pair(b, g, i0, i1, xt, wgt, qTg):
                """Two Q units with db-interleaved matmuls: during the
                DMA-gated startup the PE gets two matmuls per arriving
                xt d-block instead of one."""
                gsfx = f"_b{b}_g{g}"
                wt = wgt["wq"]
                e0 = g * GW
                pair = []
                for i in (i0, i1):
                    eb, nch = divmod(i, NC2)
                    acc = ps.tile([P, 512], F32, tag="pj", bufs=2,
                                  name=f"pq{gsfx}_{eb}_{nch}")
                    pair.append((eb, nch, acc))
                for db in range(DB):
                    for eb, nch, acc in pair:
                        nc.tensor.matmul(
                            acc,
                            r(wt[:, db, eb * P:(eb + 1) * P]),
                            r(xt[:, db, nch * 512:(nch + 1) * 512]),
                            start=(db == 0), stop=(db == DB - 1))
                for eb, nch, acc in pair:
                    ebg = (e0 // P) + eb
                    if has_bias:
                        nc.vector.tensor_scalar_add(
                            qTg[:, eb, nch * 512:(nch + 1) * 512],
                            acc, bq_col[:, ebg:ebg + 1])
                    else:
                        nc.vector.tensor_copy(
                            qTg[:, eb, nch * 512:(nch + 1) * 512], acc)

            def emit_proj_unit(b, g, kind, idx, xt, wgt, qTg, kTg, vg):
                """Emit one psum accumulation group of phase 1."""
                gsfx = f"_b{b}_g{g}"
                e0 = g * GW
                if kind in ("q", "k"):
                    dst, wt, bcol = ((qTg, wgt["wq"], bq_col) if kind == "q"
                                     else (kTg, wgt["wk"], bk_col))
                    eb, nch = divmod(idx, NC2)
                    acc = ps.tile([P, 512], F32, tag="pj", bufs=2,
                                  name=f"p{kind}{gsfx}_{eb}_{nch}")
                    for db in range(DB):
                        nc.tensor.matmul(
                            acc,
                            r(wt[:, db, eb * P:(eb + 1) * P]),
                            r(xt[:, db, nch * 512:(nch + 1) * 512]),
                            start=(db == 0), stop=(db == DB - 1))
                    ebg = (e0 // P) + eb
                    if has_bias:
                        nc.vector.tensor_scalar_add(
                            dst[:, eb, nch * 512:(nch + 1) * 512],
                            acc, bcol[:, ebg:ebg + 1])
                    else:
                        nc.vector.tensor_copy(
                            dst[:, eb, nch * 512:(nch + 1) * 512], acc)
                else:  # "v"
                    nb = idx
                    accv = ps.tile([P, 512], F32, tag="pj", bufs=2,
                                   name=f"pv{gsfx}_{nb}")
                    for db in range(DB):
                        nc.tensor.matmul(
                            accv[:, :GW],
                            r(xt[:, db, nb * P:(nb + 1) * P]),
                            r(wgt["wv"][:, db, :]),
                            start=(db == 0), stop=(db == DB - 1))
                    nc.vector.tensor_copy(vg[:, nb, :], accv[:, :GW])

            def make_phase3(b, hT):
                """Return list of emit closures: [c-prelude?] + 16 po units
                ordered so the first 8 only need q rows < 512 (qc=0)."""
                units = []
                sfx = f"_b{b}"

                if b == 0 and has_bias:
                    def emit_c():
                        for oc in range(NC2):
                            pc = ps.tile([1, 512], F32, tag="pj", bufs=2,
                                         name=f"pc_{oc}")
                            for eb in range(DB):
                                nc.tensor.matmul(pc, r(bv_col[:, eb:eb + 1]),
                                                 r(wo_sb[:, eb, oc * 512:(oc + 1) * 512]),
                                                 start=(eb == 0), stop=(eb == DB - 1))
                            nc.vector.tensor_copy(c_sb[:, oc, :], pc)
                    units.append(emit_c)

                def make_po(oc, nb):
                    def emit():
                        po = ps.tile([P, 512], F32, tag="pj", bufs=2,
                                     name=f"po{sfx}_{oc}_{nb}")
                        for eb in range(H):
                            nc.tensor.matmul(
                                po,
                                r(hT[:, eb, nb * P:(nb + 1) * P]),
                                r(wo_sb[:, eb, oc * 512:(oc + 1) * 512]),
                                start=(eb == 0),
                                stop=(not has_bias and eb == H - 1))
                        if has_bias:
                            nc.tensor.matmul(po, r(ones_row), r(c_sb[:, oc, :]),
                                             start=False, stop=True)
                        osb = work.tile([P, 512], F32, name=f"o{sfx}_{oc}_{nb}",
                                        tag="osb", bufs=2)
                        nc.scalar.activation(osb, po,
                                             mybir.ActivationFunctionType.Copy)
                        nc.sync.dma_start(
                            out=out[b, nb * P:(nb + 1) * P, oc * 512:(oc + 1) * 512],
                            in_=osb)
                    return emit

                for nb in range(DB // 2):       # q rows < 512 only
                    for oc in range(NC2):
                        units.append(make_po(oc, nb))
                for nb in range(DB // 2, DB):   # q rows >= 512
                    for oc in range(NC2):
                        units.append(make_po(oc, nb))
                return units

            def make_attn(g, hh, qc, qTg, kTg, vg, hT, b):
                """Split attention unit: (emit_scores, emit_av)."""
                h = g * GH + hh
                asfx = f"_b{b}_h{h}_q{qc}"
                st = {}
                add = mybir.AluOpType.add

                def emit_scores():
                    eT = work.tile([P, 4, 1024], BF16, name=f"eT{asfx}",
                                   tag="eT", bufs=(1 if has_bias else 2))
                    st["eT"] = eT
                    for j in range(4):
                        # scores for kb=2j, 2j+1 into one 2-bank tile
                        sp = ps.tile([P, 1024], F32, tag="spair", bufs=2,
                                     name=f"sp{asfx}_{j}")
                        for half in range(2):
                            kb = 2 * j + half
                            nc.tensor.matmul(
                                sp[:, half * 512:(half + 1) * 512],
                                r(kTg[:, hh, kb * P:(kb + 1) * P]),
                                r(qTg[:, hh, qc * 512:(qc + 1) * 512]),
                                start=True, stop=True)
                        nc.scalar.activation(
                            eT[:, j, :], sp,
                            mybir.ActivationFunctionType.Exp,
                            scale=float(NORM))

                def emit_av():
                    eT = st["eT"]
                    # heads^T (unnormalized): [dv(128) x q(512)]
                    pav = ps.tile([P, 512], F32, tag="pav", bufs=1,
                                  name=f"pav{asfx}")
                    for j in range(4):
                        for half in range(2):
                            kb = 2 * j + half
                            nc.tensor.matmul(
                                pav,
                                r(vg[:, kb, hh * DH:(hh + 1) * DH]),
                                r(eT[:, j, half * 512:(half + 1) * 512]),
                                start=(kb == 0), stop=(kb == DB - 1))

                    # R = col-sum of E^T: serial accumulate on DVE (keeps the
                    # post-exp dependency chain short: last exp -> 2 adds)
                    tA = work.tile([P, 1024], F32R, name=f"tA{asfx}", tag="tA", bufs=1)
                    rp = work.tile([P, 512], F32R, name=f"rp{asfx}", tag="rp", bufs=1)
                    nc.vector.tensor_tensor(tA, eT[:, 0, :], eT[:, 1, :], add)
                    nc.vector.tensor_tensor(tA, tA, eT[:, 2, :], add)
                    nc.vector.tensor_tensor(tA, tA, eT[:, 3, :], add)
                    nc.vector.tensor_tensor(rp, tA[:, 0:512], tA[:, 512:1024], add)
                    # colsum of rp, broadcast to all partitions, in
                    # one matmul: every row of ones128.T @ rp is R
                    pbc = ps.tile([P, 512], F32, tag="pnorm", bufs=1, name=f"pbc{asfx}")
                    nc.tensor.matmul(pbc, r(ones128), r(rp),
                                     start=True, stop=True)
                    # 1/R at full 128-lane width (approx + one NR pass)
                    # (scratch shares the rp slot: rp's only reader, the pbc
                    # matmul, always precedes the reciprocal that writes it)
                    scratch = work.tile([P, 512], F32, name=f"sc{asfx}",
                                        tag="rp", bufs=1)
                    binv = work.tile([P, 512], F32, name=f"binv{asfx}",
                                     tag="binv", bufs=1)
                    nc.vector.reciprocal_approx_accurate(binv, pbc, scratch)
                    nc.vector.tensor_tensor(
                        hT[:, h, qc * 512:(qc + 1) * 512], pav, binv,
                        mybir.AluOpType.mult)

                return emit_scores, emit_av

            # attention units and the previous batch's output projection are
            # emitted interleaved with later projection units so PE always
            # has ready matmuls during exp/epilogue waits
            attn_queue = []
            pending_phase3 = None

            xt_tiles = {}

            def load_xt(b):
                xtl = big.tile([P, DB, N], BF16, name=f"xt_b{b}", tag="xt",
                               bufs=2)
                xsrc = xT[b].rearrange("(db p) n -> p db n", p=P)
                for db in range(DB):
                    nc.sync.dma_start(out=xtl[:, db, :], in_=xsrc[:, db, :])
                xt_tiles[b] = xtl

            for b in range(BPC):
                sfx = f"_b{b}"

                wgt0 = None
                if b == 0:
                    # startup: wq for group 0 must land before anything else
                    # so the first projection matmuls can begin immediately
                    wgt0 = alloc_wgt(0, 0)
                    issue_wgt_dma(wgt0, 0, names=("wq",))
                    load_xt(0)
                    issue_wgt_dma(wgt0, 0, names=("wk", "wv"))
                    emit_consts()
                xt = xt_tiles[b]

                hT = None

                for g in range(G):
                    # ---- weight slices for this group: [128, db, GW]
                    if b == 0 and g == 0:
                        wgt = wgt0
                    else:
                        wgt = alloc_wgt(b, g)
                        issue_wgt_dma(wgt, g)
                    if b == 0 and g == 1:
                        issue_wo_dma()
                    if g == G - 1 and b + 1 < BPC:
                        # prefetch the next batch's x^T while this batch's
                        # last attention/projection work is still running
                        load_xt(b + 1)

                    qTg = work.tile([P, GH, N], BF16, name=f"qT{sfx}_g{g}", tag="qTg", bufs=2)
                    kTg = work.tile([P, GH, N], BF16, name=f"kT{sfx}_g{g}", tag="kTg", bufs=2)
                    vg = work.tile([P, DB, GW], BF16, name=f"v{sfx}_g{g}", tag="vg", bufs=2)

                    # 16 proj units: 4 Q, 4 K, 8 V; interleave with up to 4
                    # pending attention units (1 attention per 4 proj units)
                    if b == 0 and g == 0:
                        emit_q_pair(b, g, 0, 1, xt, wgt, qTg)
                        emit_q_pair(b, g, 2, 3, xt, wgt, qTg)
                        units = ([("k", i) for i in range(GH * NC2)]
                                 + [("v", i) for i in range(DB)])
                    else:
                        units = ([("q", i) for i in range(GH * NC2)]
                                 + [("k", i) for i in range(GH * NC2)]
                                 + [("v", i) for i in range(DB)])
                    for ui, (kind, idx) in enumerate(units):
                        emit_proj_unit(b, g, kind, idx, xt, wgt, qTg, kTg, vg)
                        if ui % 4 == 1 and attn_queue:
                            s_fn, a_fn = attn_queue.pop(0)
                            s_fn(); a_fn()
                    while attn_queue:
                        s_fn, a_fn = attn_queue.pop(0)
                        s_fn(); a_fn()
                    if pending_phase3 is not None:
                        for u in pending_phase3:
                            u()
                        pending_phase3 = None
                    if hT is None:
                        hT = big.tile([P, H, N], BF16, name=f"hT{sfx}", tag="hT")

                    # ---- queue attention for the heads of this group,
                    # q-chunk-major so all heads' qc=0 results land first
                    for qc in range(NC2):
                        for hh in range(GH):
                            attn_queue.append(
                                make_attn(g, hh, qc, qTg, kTg, vg, hT, b))

                if b < BPC - 1:
                    # phase 3 of this batch is deferred: it is emitted after
                    # the next batch's first projection group so its matmuls
                    # overlap the last attention units
                    pending_phase3 = make_phase3(b, hT)
                else:
                    # final batch: interleave the last group's attention with
                    # the output projection so PE keeps working through the
                    # exp chains of the final units
                    p3 = make_phase3(b, hT)
                    pre = p3[:-16]        # c-prelude if present (b==0 case)
                    po = p3[-16:]         # po[:8] need qc=0 only
                    drain = list(attn_queue)
                    attn_queue = []
                    # qc=0 units (first half of queue): run normally
                    for s_fn, a_fn in drain[:-2]:
                        s_fn(); a_fn()
                    for u in pre:
                        u()
                    # last two (qc=1) units: fill exp latency with po units
                    (s0, a0), (s1, a1) = drain[-2:]
                    s0()
                    po[0](); po[1]()
                    a0()
                    s1()
                    po[2](); po[3]()
                    a1()
                    for u in po[4:]:
                        u()

            # tail: drain remaining attention, then the last output projection
            while attn_queue:
                s_fn, a_fn = attn_queue.pop(0)
                s_fn(); a_fn()
            if pending_phase3 is not None:
                for u in pending_phase3:
                    u()

    nc.compile()
    return nc


_NC_CACHE = {}


def _get_nc(has_bias):
    if has_bias not in _NC_CACHE:
        _NC_CACHE[has_bias] = build_nc(has_bias)
    return _NC_CACHE[has_bias]


def make_in_maps(x, Wq, bq, Wk, bk, Wv, bv, Wo):
    import ml_dtypes
    bf16 = ml_dtypes.bfloat16
    x = np.asarray(x, dtype=np.float32)
    in_maps = []
    shared = {
        "Wq": np.ascontiguousarray(np.asarray(Wq, dtype=np.float32).astype(bf16)),
        "Wk": np.ascontiguousarray(np.asarray(Wk, dtype=np.float32).astype(bf16)),
        "Wv": np.ascontiguousarray(np.asarray(Wv, dtype=np.float32).astype(bf16)),
        "Wo": np.ascontiguousarray(np.asarray(Wo, dtype=np.float32).astype(bf16)),
        "bq": np.ascontiguousarray(bq, dtype=np.float32),
        "bk": np.ascontiguousarray(bk, dtype=np.float32),
        "bv": np.ascontiguousarray(np.asarray(bv, dtype=np.float32).astype(bf16)),
    }
    for c in range(N_CORES):
        xc = x[c * BPC:(c + 1) * BPC]                 # [BPC, N, D]
        xTc = np.ascontiguousarray(xc.transpose(0, 2, 1).astype(bf16))
        in_maps.append({"xT": xTc, **shared})
    return in_maps


def run(x, Wq, bq, Wk, bk, Wv, bv, Wo, trace=False):
    has_bias = bool(np.any(np.asarray(bq)) or np.any(np.asarray(bk))
                    or np.any(np.asarray(bv)))
    nc = _get_nc(has_bias)
    in_maps = make_in_maps(x, Wq, bq, Wk, bk, Wv, bv, Wo)
    res = run_bass_kernel_spmd(nc, in_maps, list(range(N_CORES)), trace=trace)
    out = np.concatenate([res.results[c]["out"] for c in range(N_CORES)], axis=0)
    return out, res


def kernel(x, Wq, bq, Wk, bk, Wv, bv, Wo):
    out, _ = run(x, Wq, bq, Wk, bk, Wv, bv, Wo, trace=False)
    return out


# revision 26
# speedup vs baseline: 1.2017x; 1.0108x over previous
"""Multi-head attention (B=16, N=1024, D=1024, H=8, dh=128) on 8 trn2 cores.

Strategy: data-parallel over batch (2 batches/core), fp32r matmuls.
Per batch on each core:
  phase 1 (per 2-head group g): Q^T_g, K^T_g (head-transposed: dh on
    partitions) and V_g (natural) via fp32r matmuls from x^T (host-side
    pre-transposed) and streamed weight slices.
  phase 2 (per head, per 512-wide q chunk): S^T = K_h^T.T @ Q_h^T (k on
    partitions), E^T = exp(norm*S^T) on ACT, heads^T += V_h.T @ E^T, and
    R = colsum(E^T) via DVE/Pool pairwise adds, then one all-ones 128x128
    matmul that yields R already broadcast to every partition; 1/R via
    a fast 128-lane reciprocal, applied while copying heads^T to SBUF.
  phase 3: out = (heads_norm) @ Wo in natural layout (+ bv@Wo row via a
    K=1 matmul when biases are nonzero).

Scheduling: Wo is resident in SBUF (loaded once), startup DMAs are
ordered so the first projection matmuls start as early as possible,
attention units are queued q-chunk-major so the final batch's output
projection can interleave with the attention drain, and PSUM->SBUF
copies run on the otherwise-idle Pool engine.
"""

import numpy as np

import concourse.bass as bass
import concourse.mybir as mybir
import concourse.tile as tile
from concourse import bacc
from concourse.bass_utils import run_bass_kernel_spmd

N_CORES = 8
B = 16
BPC = B // N_CORES      # batches per core
N = 1024                # sequence length
D = 1024                # model dim
H = 8                   # heads
DH = 128                # head dim
P = 128
DB = D // P             # 8 contraction blocks
GH = 2                  # heads per group
G = H // GH             # 4 groups
GW = GH * DH            # 256: e-width per group
NC2 = N // 512          # 2 n-chunks of 512
NORM = 1.0 / np.sqrt(DH)

F32 = mybir.dt.float32
F32R = mybir.dt.float32r
BF16 = mybir.dt.bfloat16


def r(ap):
    return ap


def build_nc(has_bias=True):
    nc = bacc.Bacc()
    xT = nc.declare_dram_parameter("xT", [BPC, D, N], BF16, isOutput=False)
    Wq = nc.declare_dram_parameter("Wq", [D, D], BF16, isOutput=False)
    Wk = nc.declare_dram_parameter("Wk", [D, D], BF16, isOutput=False)
    Wv = nc.declare_dram_parameter("Wv", [D, D], BF16, isOutput=False)
    Wo = nc.declare_dram_parameter("Wo", [D, D], BF16, isOutput=False)
    bq = nc.declare_dram_parameter("bq", [D], F32, isOutput=False)
    bk = nc.declare_dram_parameter("bk", [D], F32, isOutput=False)
    bv = nc.declare_dram_parameter("bv", [D], BF16, isOutput=False)
    out = nc.declare_dram_parameter("out", [BPC, N, D], F32, isOutput=True)

    ws = [Wq, Wk, Wv]

    with tile.TileContext(nc) as tc:
        with tc.tile_pool(name="big", bufs=1) as big, \
             tc.tile_pool(name="wp", bufs=1) as wp, \
             tc.tile_pool(name="work", bufs=1) as work, \
             tc.tile_pool(name="small", bufs=1) as small, \
             tc.tile_pool(name="ps", bufs=1, space="PSUM") as ps:

            # constants / biases (tiles now; DMAs/memsets emitted after the
            # startup-critical wq/xT DMAs so those win the queue)
            bq_col = small.tile([P, DB], F32, name="bq_col")
            bk_col = small.tile([P, DB], F32, name="bk_col")
            bv_col = small.tile([P, DB], BF16, name="bv_col")
            ones128_f32 = small.tile([P, P], F32, name="ones128_f32")
            ones128 = small.tile([P, P], BF16, name="ones128")
            if has_bias:
                ones_row_f32 = small.tile([1, P], F32, name="ones_row_f32")
                ones_row = small.tile([1, P], BF16, name="ones_row")
                c_sb = small.tile([1, NC2, 512], BF16, name="c_sb")

            def emit_consts():
                if has_bias:
                    nc.sync.dma_start(out=bq_col, in_=bq.rearrange("(eb p) -> p eb", p=P))
                    nc.sync.dma_start(out=bk_col, in_=bk.rearrange("(eb p) -> p eb", p=P))
                    nc.sync.dma_start(out=bv_col, in_=bv.rearrange("(eb p) -> p eb", p=P))
                nc.vector.memset(ones128_f32, 1.0)
                nc.vector.tensor_copy(ones128, ones128_f32)
                if has_bias:
                    nc.vector.memset(ones_row_f32, 1.0)
                    nc.vector.tensor_copy(ones_row, ones_row_f32)

            # persistent Wo: [128, eb, o] loaded once, shared by both batches
            wo_sb = wp.tile([P, DB, N], BF16, name="wo_sb", tag="wo_sb")
            wo_src = Wo.rearrange("(eb p) o -> p eb o", p=P)

            def issue_wo_dma():
                for q in range(4):
                    nc.sync.dma_start(out=wo_sb[:, 2 * q:2 * q + 2, :],
                                      in_=wo_src[:, 2 * q:2 * q + 2, :])

            def alloc_wgt(b, g):
                gsfx = f"_b{b}_g{g}"
                wgt = {}
                for wname in ("wq", "wk", "wv"):
                    wgt[wname] = wp.tile([P, DB, GW], BF16, name=f"{wname}{gsfx}",
                                         tag="wg", bufs=2)
                return wgt

            def issue_wgt_dma(wgt, g, names=("wq", "wk", "wv")):
                e0 = g * GW
                for wi, wname in enumerate(("wq", "wk", "wv")):
                    if wname not in names:
                        continue
                    wt = wgt[wname]
                    src = ws[wi].rearrange("(db p) e -> p db e", p=P)
                    nc.sync.dma_start(out=wt[:, 0:DB // 2, :],
                                      in_=src[:, 0:DB // 2, e0:e0 + GW])
                    nc.sync.dma_start(out=wt[:, DB // 2:, :],
                                      in_=src[:, DB // 2:, e0:e0 + GW])

            def emit_q_pair(b, g, i0, i1, xt, wgt, qTg):
                """Two Q units with db-interleaved matmuls: during the
                DMA-gated startup the PE gets two matmuls per arriving
                xt d-block instead of one."""
                gsfx = f"_b{b}_g{g}"
                wt = wgt["wq"]
                e0 = g * GW
                pair = []
                for i in (i0, i1):
                    eb, nch = divmod(i, NC2)
                    acc = ps.tile([P, 512], F32, tag="pj", bufs=2,
                                  name=f"pq{gsfx}_{eb}_{nch}")
                    pair.append((eb, nch, acc))
                for db in range(DB):
                    for eb, nch, acc in pair:
                        nc.tensor.matmul(
                            acc,
                            r(wt[:, db, eb * P:(eb + 1) * P]),
                            r(xt[:, db, nch * 512:(nch + 1) * 512]),
                            start=(db == 0), stop=(db == DB - 1))
                for eb, nch, acc in pair:
                    ebg = (e0 // P) + eb
                    if has_bias:
                        nc.vector.tensor_scalar_add(
                            qTg[:, eb, nch * 512:(nch + 1) * 512],
                            acc, bq_col[:, ebg:ebg + 1])
                    else:
                        nc.vector.tensor_copy(
                            qTg[:, eb, nch * 512:(nch + 1) * 512], acc)

            def emit_proj_unit(b, g, kind, idx, xt, wgt, qTg, kTg, vg):
                """Emit one psum accumulation group of phase 1."""
                gsfx = f"_b{b}_g{g}"
                e0 = g * GW
                if kind in ("q", "k"):
                    dst, wt, bcol = ((qTg, wgt["wq"], bq_col) if kind == "q"
                                     else (kTg, wgt["wk"], bk_col))
                    eb, nch = divmod(idx, NC2)
                    acc = ps.tile([P, 512], F32, tag="pj", bufs=2,
                                  name=f"p{kind}{gsfx}_{eb}_{nch}")
                    for db in range(DB):
                        nc.tensor.matmul(
                            acc,
                            r(wt[:, db, eb * P:(eb + 1) * P]),
                            r(xt[:, db, nch * 512:(nch + 1) * 512]),
                            start=(db == 0), stop=(db == DB - 1))
                    ebg = (e0 // P) + eb
                    if has_bias:
                        nc.vector.tensor_scalar_add(
                            dst[:, eb, nch * 512:(nch + 1) * 512],
                            acc, bcol[:, ebg:ebg + 1])
                    else:
                        nc.vector.tensor_copy(
                            dst[:, eb, nch * 512:(nch + 1) * 512], acc)
                else:  # "v"
                    nb = idx
                    accv = ps.tile([P, 512], F32, tag="pj", bufs=2,
                                   name=f"pv{gsfx}_{nb}")
                    for db in range(DB):
                        nc.tensor.matmul(
                            accv[:, :GW],
                            r(xt[:, db, nb * P:(nb + 1) * P]),
                            r(wgt["wv"][:, db, :]),
                            start=(db == 0), stop=(db == DB - 1))
                    nc.vector.tensor_copy(vg[:, nb, :], accv[:, :GW])

            def make_phase3(b, hT):
                """Return list of emit closures: [c-prelude?] + 16 po units
                ordered so the first 8 only need q rows < 512 (qc=0)."""
                units = []
                sfx = f"_b{b}"

                if b == 0 and has_bias:
                    def emit_c():
                        for oc in range(NC2):
                            pc = ps.tile([1, 512], F32, tag="pj", bufs=2,
                                         name=f"pc_{oc}")
                            for eb in range(DB):
                                nc.tensor.matmul(pc, r(bv_col[:, eb:eb + 1]),
                                                 r(wo_sb[:, eb, oc * 512:(oc + 1) * 512]),
                                                 start=(eb == 0), stop=(eb == DB - 1))
                            nc.vector.tensor_copy(c_sb[:, oc, :], pc)
                    units.append(emit_c)

                def make_po(oc, nb):
                    def emit():
                        po = ps.tile([P, 512], F32, tag="pj", bufs=2,
                                     name=f"po{sfx}_{oc}_{nb}")
                        for eb in range(H):
                            nc.tensor.matmul(
                                po,
                                r(hT[:, eb, nb * P:(nb + 1) * P]),
                                r(wo_sb[:, eb, oc * 512:(oc + 1) * 512]),
                                start=(eb == 0),
                                stop=(not has_bias and eb == H - 1))
                        if has_bias:
                            nc.tensor.matmul(po, r(ones_row), r(c_sb[:, oc, :]),
                                             start=False, stop=True)
                        osb = work.tile([P, 512], F32, name=f"o{sfx}_{oc}_{nb}",
                                        tag="osb", bufs=2)
                        nc.scalar.activation(osb, po,
                                             mybir.ActivationFunctionType.Copy)
                        nc.sync.dma_start(
                            out=out[b, nb * P:(nb + 1) * P, oc * 512:(oc + 1) * 512],
                            in_=osb)
                    return emit

                for nb in range(DB // 2):       # q rows < 512 only
                    for oc in range(NC2):
                        units.append(make_po(oc, nb))
                for nb in range(DB // 2, DB):   # q rows >= 512
                    for oc in range(NC2):
                        units.append(make_po(oc, nb))
                return units

            def make_attn(g, hh, qc, qTg, kTg, vg, hT, b):
                """Split attention unit: (emit_scores, emit_av)."""
                h = g * GH + hh
                asfx = f"_b{b}_h{h}_q{qc}"
                st = {}
                add = mybir.AluOpType.add

                def emit_scores():
                    eT = work.tile([P, 4, 1024], BF16, name=f"eT{asfx}",
                                   tag="eT", bufs=(1 if has_bias else 3))
                    st["eT"] = eT
                    for j in range(4):
                        # scores for kb=2j, 2j+1 into one 2-bank tile
                        sp = ps.tile([P, 1024], F32, tag="spair", bufs=2,
                                     name=f"sp{asfx}_{j}")
                        for half in range(2):
                            kb = 2 * j + half
                            nc.tensor.matmul(
                                sp[:, half * 512:(half + 1) * 512],
                                r(kTg[:, hh, kb * P:(kb + 1) * P]),
                                r(qTg[:, hh, qc * 512:(qc + 1) * 512]),
                                start=True, stop=True)
                        nc.scalar.activation(
                            eT[:, j, :], sp,
                            mybir.ActivationFunctionType.Exp,
                            scale=float(NORM))

                def emit_av():
                    eT = st["eT"]
                    # heads^T (unnormalized): [dv(128) x q(512)]
                    pav = ps.tile([P, 512], F32, tag="pav", bufs=1,
                                  name=f"pav{asfx}")
                    for j in range(4):
                        for half in range(2):
                            kb = 2 * j + half
                            nc.tensor.matmul(
                                pav,
                                r(vg[:, kb, hh * DH:(hh + 1) * DH]),
                                r(eT[:, j, half * 512:(half + 1) * 512]),
                                start=(kb == 0), stop=(kb == DB - 1))

                    # R = col-sum of E^T: serial accumulate on DVE (keeps the
                    # post-exp dependency chain short: last exp -> 2 adds)
                    tA = work.tile([P, 1024], F32R, name=f"tA{asfx}", tag="tA", bufs=1)
                    rp = work.tile([P, 512], BF16, name=f"rp{asfx}", tag="rp", bufs=1)
                    nc.vector.tensor_tensor(tA, eT[:, 0, :], eT[:, 1, :], add)
                    nc.vector.tensor_tensor(tA, tA, eT[:, 2, :], add)
                    nc.vector.tensor_tensor(tA, tA, eT[:, 3, :], add)
                    nc.vector.tensor_tensor(rp, tA[:, 0:512], tA[:, 512:1024], add)
                    # colsum of rp, broadcast to all partitions, in
                    # one matmul: every row of ones128.T @ rp is R
                    pbc = ps.tile([P, 512], F32, tag="pnorm", bufs=1, name=f"pbc{asfx}")
                    nc.tensor.matmul(pbc, r(ones128), r(rp),
                                     start=True, stop=True)
                    # 1/R at full 128-lane width (approx + one NR pass)
                    # (scratch shares the rp slot: rp's only reader, the pbc
                    # matmul, always precedes the reciprocal that writes it)
                    scratch = work.tile([P, 512], F32, name=f"sc{asfx}",
                                        tag="rp", bufs=1)
                    binv = work.tile([P, 512], F32, name=f"binv{asfx}",
                                     tag="binv", bufs=1)
                    nc.vector.reciprocal_approx_accurate(binv, pbc, scratch)
                    nc.vector.tensor_tensor(
                        hT[:, h, qc * 512:(qc + 1) * 512], pav, binv,
                        mybir.AluOpType.mult)

                return emit_scores, emit_av

            # attention units and the previous batch's output projection are
            # emitted interleaved with later projection units so PE always
            # has ready matmuls during exp/epilogue waits
            attn_queue = []
            pending_phase3 = None

            xt_tiles = {}

            def load_xt(b):
                xtl = big.tile([P, DB, N], BF16, name=f"xt_b{b}", tag="xt",
                               bufs=2)
                xsrc = xT[b].rearrange("(db p) n -> p db n", p=P)
                for db in range(DB):
                    nc.sync.dma_start(out=xtl[:, db, :], in_=xsrc[:, db, :])
                xt_tiles[b] = xtl

            for b in range(BPC):
                sfx = f"_b{b}"

                wgt0 = None
                if b == 0:
                    # startup: wq for group 0 must land before anything else
                    # so the first projection matmuls can begin immediately
                    wgt0 = alloc_wgt(0, 0)
                    issue_wgt_dma(wgt0, 0, names=("wq",))
                    load_xt(0)
                    issue_wgt_dma(wgt0, 0, names=("wk", "wv"))
                    emit_consts()
                xt = xt_tiles[b]

                hT = None

                for g in range(G):
                    # ---- weight slices for this group: [128, db, GW]
                    if b == 0 and g == 0:
                        wgt = wgt0
                    else:
                        wgt = alloc_wgt(b, g)
                        issue_wgt_dma(wgt, g)
                    if b == 0 and g == 1:
                        issue_wo_dma()
                    if g == G - 1 and b + 1 < BPC:
                        # prefetch the next batch's x^T while this batch's
                        # last attention/projection work is still running
                        load_xt(b + 1)

                    qTg = work.tile([P, GH, N], BF16, name=f"qT{sfx}_g{g}", tag="qTg", bufs=2)
                    kTg = work.tile([P, GH, N], BF16, name=f"kT{sfx}_g{g}", tag="kTg", bufs=2)
                    vg = work.tile([P, DB, GW], BF16, name=f"v{sfx}_g{g}", tag="vg", bufs=2)

                    # 16 proj units: 4 Q, 4 K, 8 V; interleave with up to 4
                    # pending attention units (1 attention per 4 proj units)
                    if b == 0 and g == 0:
                        emit_q_pair(b, g, 0, 1, xt, wgt, qTg)
                        emit_q_pair(b, g, 2, 3, xt, wgt, qTg)
                        units = ([("k", i) for i in range(GH * NC2)]
                                 + [("v", i) for i in range(DB)])
                    else:
                        units = ([("q", i) for i in range(GH * NC2)]
                                 + [("k", i) for i in range(GH * NC2)]
                                 + [("v", i) for i in range(DB)])
                    hold = 2 if (b == BPC - 1 and g == G - 1) else 0
                    for ui, (kind, idx) in enumerate(units):
                        emit_proj_unit(b, g, kind, idx, xt, wgt, qTg, kTg, vg)
                        if ui % 4 == 1 and len(attn_queue) > hold:
                            s_fn, a_fn = attn_queue.pop(0)
                            s_fn(); a_fn()
                    while len(attn_queue) > hold:
                        s_fn, a_fn = attn_queue.pop(0)
                        s_fn(); a_fn()
                    if pending_phase3 is not None:
                        for u in pending_phase3:
                            u()
                        pending_phase3 = None
                    if hT is None:
                        hT = big.tile([P, H, N], BF16, name=f"hT{sfx}", tag="hT")

                    # ---- queue attention for the heads of this group,
                    # q-chunk-major so all heads' qc=0 results land first
                    for qc in range(NC2):
                        for hh in range(GH):
                            attn_queue.append(
                                make_attn(g, hh, qc, qTg, kTg, vg, hT, b))

                if b < BPC - 1:
                    # phase 3 of this batch is deferred: it is emitted after
                    # the next batch's first projection group so its matmuls
                    # overlap the last attention units
                    pending_phase3 = make_phase3(b, hT)
                else:
                    # final batch: interleave the last group's attention with
                    # the output projection so PE keeps working through the
                    # exp chains of the final units
                    p3 = make_phase3(b, hT)
                    pre = p3[:-16]        # c-prelude if present (b==0 case)
                    po = p3[-16:]         # po[:8] need qc=0 only
                    drain = list(attn_queue)
                    attn_queue = []
                    lead = drain[:-4]     # held-back prev-group units: free
                    (sA, aA), (sB, aB), (sC, aC), (sD, aD) = drain[-4:]
                    # weave the free units between the exp-bound qc0 units
                    sA()
                    if len(lead) > 0:
                        lead[0][0]()
                    aA()
                    if len(lead) > 0:
                        lead[0][1]()
                    sB()
                    if len(lead) > 1:
                        lead[1][0]()
                    aB()
                    if len(lead) > 1:
                        lead[1][1]()
                    for u in pre:
                        u()
                    for s_fn, a_fn in lead[2:]:
                        s_fn(); a_fn()
                    # last two (qc=1) units: fill exp latency with po units
                    sC()
                    po[0](); po[1]()
                    aC()
                    sD()
                    po[2](); po[3]()
                    aD()
                    for u in po[4:]:
                        u()

            # tail: drain remaining attention, then the last output projection
            while attn_queue:
                s_fn, a_fn = attn_queue.pop(0)
                s_fn(); a_fn()
            if pending_phase3 is not None:
                for u in pending_phase3:
                    u()

    nc.compile()
    return nc


_NC_CACHE = {}


def _get_nc(has_bias):
    if has_bias not in _NC_CACHE:
        _NC_CACHE[has_bias] = build_nc(has_bias)
    return _NC_CACHE[has_bias]


def make_in_maps(x, Wq, bq, Wk, bk, Wv, bv, Wo):
    import ml_dtypes
    bf16 = ml_dtypes.bfloat16
    x = np.asarray(x, dtype=np.float32)
    in_maps = []
    shared = {
        "Wq": np.ascontiguousarray(np.asarray(Wq, dtype=np.float32).astype(bf16)),
        "Wk": np.ascontiguousarray(np.asarray(Wk, dtype=np.float32).astype(bf16)),
        "Wv": np.ascontiguousarray(np.asarray(Wv, dtype=np.float32).astype(bf16)),
        "Wo": np.ascontiguousarray(np.asarray(Wo, dtype=np.float32).astype(bf16)),
        "bq": np.ascontiguousarray(bq, dtype=np.float32),
        "bk": np.ascontiguousarray(bk, dtype=np.float32),
        "bv": np.ascontiguousarray(np.asarray(bv, dtype=np.float32).astype(bf16)),
    }
    for c in range(N_CORES):
        xc = x[c * BPC:(c + 1) * BPC]                 # [BPC, N, D]
        xTc = np.ascontiguousarray(xc.transpose(0, 2, 1).astype(bf16))
        in_maps.append({"xT": xTc, **shared})
    return in_maps


def run(x, Wq, bq, Wk, bk, Wv, bv, Wo, trace=False):
    has_bias = bool(np.any(np.asarray(bq)) or np.any(np.asarray(bk))
                    or np.any(np.asarray(bv)))
    nc = _get_nc(has_bias)
    in_maps = make_in_maps(x, Wq, bq, Wk, bk, Wv, bv, Wo)
    res = run_bass_kernel_spmd(nc, in_maps, list(range(N_CORES)), trace=trace)
    out = np.concatenate([res.results[c]["out"] for c in range(N_CORES)], axis=0)
    return out, res


def kernel(x, Wq, bq, Wk, bk, Wv, bv, Wo):
    out, _ = run(x, Wq, bq, Wk, bk, Wv, bv, Wo, trace=False)
    return out
